# revision 6
# baseline (speedup 1.0000x reference)
"""Trainium2 Bass kernel for nn_CodecAttention (sliding-window ALiBi attention).

Reference computation (B=4, T=2048, DIM=1024, H=8, HD=128, WINDOW=16):
    xq = rms_norm(x @ wq) ; xk = rms_norm(x @ wk) ; xv = x @ wv
    scores = q k^T / sqrt(HD) + alibi_bias  (causal + 16-token sliding window)
    out = softmax(scores) @ v  -> reshape -> @ wo

Sharding: 8 cores = (batch b, sequence half). Each core processes 1024 query
tokens plus a 16-token key/value halo (zeros for the first half), fully
locally -- no collectives.

v2 design notes:
  - All four projections run as fp8e4 DoubleRow matmuls with a 3-chain hi/lo
    decomposition (x_hi*w_hi + x_lo*w_hi + x_hi*w_lo): 12 DR instructions per
    128-deep-by-8 contraction instead of 8 bf16 ones (0.75x PE cost) at
    bf16-level accuracy (residual quantization error ~1e-4 per operand).
    Weight hi parts are pre-scaled per tensor (SC_*) to stay in fp8 normal
    range; the scale is folded into the RMS rsqrt (q/k) or drain scale (v/wo).
  - RMS norm: drain PSUM -> bf16, squares via one DVE stt per chunk, then a
    chained ones[128,128] matmul accumulates across the 8 feature tiles AND
    broadcasts the per-token sum to all partitions; ACT Sqrt (scale folds
    SC^2, bias = eps*SC^2) + DVE reciprocal + 8 per-chunk multiplies.
  - attention per (head, 128-query tile): ALiBi+mask bias (log domain,
    masked = -30000 -> exp gives exact 0) is preloaded into PSUM via a
    144-wide ident matmul, QK accumulates on top; ACT Exp drains PSUM->SBUF
    with accum_out producing the softmax denominator for free; DVE
    reciprocal + Pool per-partition normalize; the probs transpose runs on
    the DMA xbar (dma_start_transpose of a zero-padded [128,256] tile ->
    [128,2,128]), skipping both the PE transposes and the PSUM drain; PV in
    bf16; attention output drained as fp8 hi (ACT) + lo (DVE stt) pair.
  - output projection: 3-chain fp8 DR over (aT_hi+aT_lo)@wo_hi + aT_hi@wo_lo,
    drained with the 1/SC_O scale folded in, DMA straight to DRAM.
"""

import math
import os

import numpy as np
import ml_dtypes

os.environ.setdefault("MYCRO_LOCAL_CACHE", "1")

import concourse.mybir as mybir
import concourse.tile as tile
from concourse import bacc
from concourse.bass_utils import run_bass_kernel_spmd

F32 = mybir.dt.float32
BF16 = mybir.dt.bfloat16
F8 = mybir.dt.float8e4
AF = mybir.ActivationFunctionType
ALU = mybir.AluOpType
DR = mybir.MatmulPerfMode.DoubleRow

B, T, DIM = 4, 2048, 1024
H, HD = 8, 128
WINDOW = 16
EPS = 1e-6

HALO = 16                  # key/value halo tokens per shard
TSH = HALO + T // 2        # 1040 k/v tokens per shard
QTOK = T // 2              # 1024 query tokens per shard
ND = DIM // 128            # 8 dim tiles
NVT = 9                    # v token tiles (8*128 + 16)
NQT = QTOK // 128          # 8 query tiles
KW = 128 + HALO            # 144 keys per query tile

SC_Q = 256.0               # fp8 pre-scale for wq
SC_K = 2048.0              # fp8 pre-scale for wk*u (u ~ 1/11.3 folded in)
SC_V = 256.0               # fp8 pre-scale for wv
SC_O = 256.0               # fp8 pre-scale for wo

NEG_MASK = -30000.0        # log-domain mask; exp() underflows to exact 0

_SLOPES = [2.0 ** (-i) for i in range(H)]

_CACHE = {}


def _build_program():
    nc = bacc.Bacc("TRN2", debug=False, target_bir_lowering=False, num_devices=8)

    xh = nc.declare_dram_parameter("xh", [128, ND, TSH], F8, isOutput=False)
    xl = nc.declare_dram_parameter("xl", [128, ND, TSH], F8, isOutput=False)
    w_in = {}
    for wn in ("wq", "wk", "wv", "wo"):
        for p in ("h", "l"):
            w_in[wn + p] = nc.declare_dram_parameter(
                wn + p, [128, ND, DIM], F8, isOutput=False)
    invu2 = nc.declare_dram_parameter("invu2", [128, ND], F32, isOutput=False)
    ident = nc.declare_dram_parameter("ident", [128, 128], BF16, isOutput=False)
    logb = nc.declare_dram_parameter("logb", [128, H, 2, KW], BF16,
                                     isOutput=False)
    out = nc.declare_dram_parameter("out", [QTOK, DIM], F32, isOutput=True)

    with tile.TileContext(nc) as tc:
        with tc.tile_pool(name="big", bufs=1) as big:
            kt_sb = big.tile([128, ND, TSH], BF16)
            qt_sb = big.tile([128, ND, QTOK], BF16)
            v_sb = big.tile([128, NVT, DIM], BF16)
            woh_sb = big.tile([128, ND, DIM], F8)
            wol_sb = big.tile([128, ND, DIM], F8)
            invu2_sb = big.tile([128, ND], F32)
            ident_sb = big.tile([128, 128], BF16)
            logb_sb = big.tile([128, H, 2, KW], BF16)
            ones_sb = big.tile([128, 128], BF16)
            epsk_sb = big.tile([128, 1], F32)
            epsq_sb = big.tile([128, 1], F32)
            nc.vector.memset(ones_sb[:], 1.0)
            nc.vector.memset(epsk_sb[:], EPS * SC_K * SC_K)
            nc.vector.memset(epsq_sb[:], EPS * SC_Q * SC_Q)

            self_phase1(tc, nc, kt_sb, qt_sb, v_sb, woh_sb, wol_sb, invu2_sb,
                        ident_sb, logb_sb, ones_sb, epsk_sb, epsq_sb,
                        xh, xl, w_in, invu2, ident, logb)
            self_phase2(tc, nc, kt_sb, qt_sb, v_sb, woh_sb, wol_sb, ident_sb,
                        logb_sb, out)
    nc.compile()
    return nc


def self_phase1(tc, nc, kt_sb, qt_sb, v_sb, woh_sb, wol_sb, invu2_sb,
                ident_sb, logb_sb, ones_sb, epsk_sb, epsq_sb,
                xh, xl, w_in, invu2, ident, logb):
    with (
        tc.tile_pool(name="xtp", bufs=1) as xtp,
        tc.tile_pool(name="wp", bufs=1) as wp,
        tc.tile_pool(name="sq", bufs=int(os.environ.get("KP_SQ", 10))) as sqp,
        tc.tile_pool(name="rst", bufs=4) as rstp,
        tc.tile_pool(name="pp", bufs=int(os.environ.get("KP_PP", 4)),
                     space="PSUM") as pp,
        tc.tile_pool(name="ssp", bufs=int(os.environ.get("KP_SSP", 2)),
                     space="PSUM") as ssp,
    ):
        xh_sb = xtp.tile([128, ND, TSH], F8)
        xl_sb = xtp.tile([128, ND, TSH], F8)
        wkh_sb = wp.tile([128, ND, DIM], F8)
        wkl_sb = wp.tile([128, ND, DIM], F8)
        wqh_sb = wp.tile([128, ND, DIM], F8)
        wql_sb = wp.tile([128, ND, DIM], F8)
        wvh_sb = wp.tile([128, ND, DIM], F8)
        wvl_sb = wp.tile([128, ND, DIM], F8)

        # stream k weights + x first (k runs first)
        for kk in range(ND):
            nc.sync.dma_start(wkh_sb[:, kk, :], w_in["wkh"][:, kk, :])
            nc.sync.dma_start(xh_sb[:, kk, :], xh[:, kk, :])
            nc.sync.dma_start(wkl_sb[:, kk, :], w_in["wkl"][:, kk, :])
            nc.sync.dma_start(xl_sb[:, kk, :], xl[:, kk, :])
        nc.sync.dma_start(invu2_sb[:], invu2[:])
        nc.sync.dma_start(ident_sb[:], ident[:])

        def proj_chains(ps, wh_sb, wl_sb, m, c0, cw):
            """12 DR matmuls: xh*wh + xl*wh + xh*wl accumulated in psum."""
            for ci, (mov, sta) in enumerate(
                    [(xh_sb, wh_sb), (xl_sb, wh_sb), (xh_sb, wl_sb)]):
                for j in range(ND // 2):
                    nc.tensor.matmul(
                        ps[:, :cw],
                        sta[:, 2 * j:2 * j + 2, m * 128:(m + 1) * 128],
                        mov[:, 2 * j:2 * j + 2, c0:c0 + cw],
                        start=(ci == 0 and j == 0),
                        stop=(ci == 2 and j == ND // 2 - 1),
                        perf_mode=DR)

        def rms_chunk(dst, wh_sb, wl_sb, src_c0, dst_c0, cw, sq_scal, eps_sb,
                      sc2, di):
            """Project + rms-normalize one token chunk (src col in xh/xl
            space, dst col in the projection tile)."""
            sq_list = []
            for m in range(ND):
                ps = pp.tile([128, 512], F32, tag="ps", name="ps")
                proj_chains(ps, wh_sb, wl_sb, m, src_c0, cw)
                d = dst[:, m, dst_c0:dst_c0 + cw]
                if (di + m) % 2 == 0:
                    nc.scalar.copy(d, ps[:, :cw])
                else:
                    nc.vector.tensor_copy(d, ps[:, :cw])
                sq = sqp.tile([128, 512], BF16, tag="sq", name="sq")
                scal = sq_scal if isinstance(sq_scal, float) \
                    else sq_scal[:, m:m + 1]
                nc.vector.scalar_tensor_tensor(
                    out=sq[:, :cw], in0=d, scalar=scal,
                    in1=d, op0=ALU.mult, op1=ALU.mult)
                sq_list.append(sq)
            ssb = ssp.tile([128, 512], F32, tag="ssb", name="ssb")
            for m in range(ND):
                nc.tensor.matmul(ssb[:, :cw], ones_sb[:], sq_list[m][:, :cw],
                                 start=(m == 0), stop=(m == ND - 1))
            rms = rstp.tile([128, 512], F32, tag="rms", name="rms")
            nc.scalar.activation(rms[:, :cw], ssb[:, :cw], AF.Sqrt,
                                 bias=eps_sb[:], scale=sc2)
            rstd = rstp.tile([128, 512], BF16, tag="rstd", name="rstd")
            with nc.allow_low_precision(reason="bf16 rstd"):
                nc.vector.reciprocal(rstd[:, :cw], rms[:, :cw])
            for m in range(ND):
                eng = nc.gpsimd if m % 2 == 0 else nc.vector
                eng.tensor_mul(dst[:, m, dst_c0:dst_c0 + cw],
                               dst[:, m, dst_c0:dst_c0 + cw], rstd[:, :cw])

        # ---- k projection + rms (token chunks incl. halo) ----
        for di, (c0, cw) in enumerate([(0, 512), (512, 512), (1024, 16)]):
            rms_chunk(kt_sb, wkh_sb, wkl_sb, c0, c0, cw, invu2_sb,
                      epsk_sb, SC_K * SC_K, di)

        # queue q/v/wo weight loads (overlap with k compute)
        for kk in range(ND):
            nc.sync.dma_start(wqh_sb[:, kk, :], w_in["wqh"][:, kk, :])
            nc.sync.dma_start(wql_sb[:, kk, :], w_in["wql"][:, kk, :])
        for kk in range(ND):
            nc.sync.dma_start(wvh_sb[:, kk, :], w_in["wvh"][:, kk, :])
            nc.sync.dma_start(wvl_sb[:, kk, :], w_in["wvl"][:, kk, :])
        for kk in range(ND):
            nc.sync.dma_start(woh_sb[:, kk, :], w_in["woh"][:, kk, :])
            nc.sync.dma_start(wol_sb[:, kk, :], w_in["wol"][:, kk, :])
        nc.sync.dma_start(logb_sb[:], logb[:])

        # ---- q projection + rms (queries exclude halo) ----
        cq = 1.0 / (SC_Q * SC_Q * DIM)
        for ci in range(2):
            rms_chunk(qt_sb, wqh_sb, wql_sb, HALO + ci * 512, ci * 512, 512,
                      cq, epsq_sb, SC_Q * SC_Q, ci)

        # ---- v projection: [token, feature] layout, drain scaled 1/SC_V ----
        for tt in range(NVT):
            tw = 128 if tt < 8 else 16
            for nn in range(2):
                ps = pp.tile([128, 512], F32, tag="ps", name="ps")
                for ci, (sta, mov) in enumerate(
                        [(xh_sb, wvh_sb), (xl_sb, wvh_sb), (xh_sb, wvl_sb)]):
                    for j in range(ND // 2):
                        nc.tensor.matmul(
                            ps[:tw, :],
                            sta[:, 2 * j:2 * j + 2, tt * 128:tt * 128 + tw],
                            mov[:, 2 * j:2 * j + 2, nn * 512:(nn + 1) * 512],
                            start=(ci == 0 and j == 0),
                            stop=(ci == 2 and j == ND // 2 - 1),
                            perf_mode=DR)
                dstv = v_sb[:tw, tt, nn * 512:(nn + 1) * 512]
                if (tt + nn) % 2 == 0:
                    nc.scalar.activation(dstv, ps[:tw, :], AF.Copy,
                                         scale=1.0 / SC_V)
                else:
                    nc.vector.tensor_scalar_mul(dstv, ps[:tw, :], 1.0 / SC_V)


def self_phase2(tc, nc, kt_sb, qt_sb, v_sb, woh_sb, wol_sb, ident_sb,
                logb_sb, out):
    with (
        tc.tile_pool(name="ex", bufs=int(os.environ.get("KP_EX", 9))) as exp_,
        tc.tile_pool(name="exn", bufs=int(os.environ.get("KP_EXN", 9))) as exnp,
        tc.tile_pool(name="ext", bufs=int(os.environ.get("KP_EXT", 6))) as extp,
        tc.tile_pool(name="rs", bufs=9) as rsp,
        tc.tile_pool(name="at", bufs=int(os.environ.get("KP_AT", 3))) as atp,
        tc.tile_pool(name="ob", bufs=3) as obp,
        tc.tile_pool(name="sq2", bufs=int(os.environ.get("KP_SQ2", 3)),
                     space="PSUM") as sqp2,
        tc.tile_pool(name="yt", bufs=int(os.environ.get("KP_YT", 3)),
                     space="PSUM") as ytp,
        tc.tile_pool(name="po", bufs=int(os.environ.get("KP_PO", 2)),
                     space="PSUM") as pop,
    ):
        NEXN = int(os.environ.get("KP_EXN", 9))

        def wo_proj(t, aTh, aTl, nn):
            ps_o = pop.tile([128, 512], F32, name="ps_o")
            for ci, (sta, mov) in enumerate(
                    [(aTh, woh_sb), (aTl, woh_sb), (aTh, wol_sb)]):
                for j in range(ND // 2):
                    nc.tensor.matmul(
                        ps_o[:],
                        sta[:, 2 * j:2 * j + 2, :],
                        mov[:, 2 * j:2 * j + 2, nn * 512:(nn + 1) * 512],
                        start=(ci == 0 and j == 0),
                        stop=(ci == 2 and j == ND // 2 - 1),
                        perf_mode=DR)
            o_sb = obp.tile([128, 512], F32, tag="osb", name="o_sb")
            if nn % 2 == 0:
                nc.vector.tensor_scalar_mul(o_sb[:], ps_o[:], 1.0 / SC_O)
            else:
                nc.scalar.activation(o_sb[:], ps_o[:], AF.Copy,
                                     scale=1.0 / SC_O)
            nc.sync.dma_start(
                out[t * 128:(t + 1) * 128, nn * 512:(nn + 1) * 512],
                o_sb[:])

        exn_init = [0]
        prev = None
        for t in range(NQT):
            aTh = atp.tile([128, ND, 128], F8, tag="aTh", name="aTh")
            aTl = atp.tile([128, ND, 128], F8, tag="aTl", name="aTl")
            var = 0 if t == 0 else 1
            for h in range(H):
                sQ_t = sqp2.tile([128, KW], F32, tag="sq", name="sQ")
                sQ = sQ_t[:, :]
                nc.tensor.matmul(sQ, ident_sb[:], logb_sb[:, h, var, :],
                                 start=True, stop=False)
                nc.tensor.matmul(sQ,
                                 qt_sb[:, h, t * 128:(t + 1) * 128],
                                 kt_sb[:, h, t * 128:t * 128 + KW],
                                 start=False, stop=True)
                ex = exp_.tile([128, KW], BF16, tag="ex", name="ex")
                rs = rsp.tile([128, 1], F32, tag="rs", name="rs")
                nc.scalar.activation(ex[:], sQ, AF.Exp, accum_out=rs[:])
                rcp = rsp.tile([128, 1], F32, tag="rcp", name="rcp")
                nc.vector.reciprocal(rcp[:], rs[:])
                exn = exnp.tile([128, 256], BF16, tag="exn", name="exn")
                if exn_init[0] < NEXN:
                    nc.vector.memset(exn[:, KW:256], 0.0)
                    exn_init[0] += 1
                if h % 2 == 0:
                    nc.gpsimd.tensor_scalar_mul(exn[:, 0:KW], ex[:], rcp[:])
                else:
                    nc.vector.tensor_scalar_mul(exn[:, 0:KW], ex[:], rcp[:])
                exsT = extp.tile([128, 2, 128], BF16, tag="exsT", name="exsT")
                nc.sync.dma_start_transpose(exsT[:], exn[:])
                yT_t = ytp.tile([128, 128], F32, tag="yT", name="yT")
                yT = yT_t[:, :]
                hs = slice(h * 128, (h + 1) * 128)
                nc.tensor.matmul(yT, v_sb[:, t, hs], exsT[:, 0, :],
                                 start=True, stop=False)
                nc.tensor.matmul(yT, v_sb[0:16, t + 1, hs],
                                 exsT[0:16, 1, :], start=False, stop=True)
                with nc.allow_low_precision(reason="fp8 hi/lo attention out"):
                    nc.scalar.copy(aTh[:, h, :], yT)
                    nc.vector.scalar_tensor_tensor(
                        out=aTl[:, h, :], in0=yT, scalar=1.0,
                        in1=aTh[:, h, :], op0=ALU.mult, op1=ALU.subtract)
                if prev is not None and h in (1, 5):
                    wo_proj(t - 1, prev[0], prev[1], h // 4)
            prev = (aTh, aTl)
        wo_proj(NQT - 1, prev[0], prev[1], 0)
        wo_proj(NQT - 1, prev[0], prev[1], 1)


def _host_constants():
    # logb[i, c] = slope * (c - i - 16) inside the band (-16 <= c-i-16 <= 0),
    # else NEG_MASK.  Variant 0 additionally masks kt cols < 16 (halo before
    # sequence start).
    ii = np.arange(128)[:, None]
    cc = np.arange(KW)[None, :]
    rel = cc - ii - HALO
    band = (rel <= 0) & (rel >= -WINDOW)
    logb = np.full((128, H, 2, KW), NEG_MASK, dtype=np.float32)
    for h in range(H):
        pat = np.where(band, _SLOPES[h] * rel, NEG_MASK)
        logb[:, h, 1, :] = pat
        logb[:, h, 0, :] = np.where(cc < HALO, NEG_MASK, pat)
    ident = np.eye(128, dtype=np.float32)
    return logb, ident


def _split8(a):
    """fp8 hi/lo split of a float32 array."""
    f8 = ml_dtypes.float8_e4m3
    hi = a.astype(f8)
    lo = (a - hi.astype(np.float32)).astype(f8)
    return hi, lo


def _wlayout(w):
    """[DIM, DIM] -> [128, ND, DIM] (partition = row % 128, dim1 = row // 128)."""
    return np.ascontiguousarray(
        w.reshape(ND, 128, w.shape[1]).transpose(1, 0, 2))


def _make_in_maps(x, wq, wk, wv, wo, q_norm_w, k_norm_w):
    x = np.asarray(x, dtype=np.float32)
    wq = np.asarray(wq, dtype=np.float32)
    wk = np.asarray(wk, dtype=np.float32)
    wv = np.asarray(wv, dtype=np.float32)
    wo = np.asarray(wo, dtype=np.float32)
    q_norm_w = np.asarray(q_norm_w, dtype=np.float32)
    k_norm_w = np.asarray(k_norm_w, dtype=np.float32)

    u = (q_norm_w * k_norm_w / math.sqrt(HD)).astype(np.float32)
    wqh, wql = _split8(_wlayout(wq * SC_Q))
    wkh, wkl = _split8(_wlayout(wk * u[None, :] * SC_K))
    wvh, wvl = _split8(_wlayout(wv * SC_V))
    woh, wol = _split8(_wlayout(wo * SC_O))
    # raw sum-of-squares correction: mean_f k_raw^2 = sum_f k''^2 * invu2
    invu2 = np.ascontiguousarray(
        (1.0 / (u * u * SC_K * SC_K * DIM)).reshape(ND, 128).T
        .astype(np.float32))

    logb, ident = _host_constants()
    ident_b = ident.astype(ml_dtypes.bfloat16)

    in_maps = []
    for c in range(8):
        b, hf = c // 2, c % 2
        base = hf * (T // 2)
        xsh = np.zeros((TSH, DIM), dtype=np.float32)
        lo = base - HALO
        if lo < 0:
            xsh[HALO:] = x[b, base: base + QTOK]
        else:
            xsh[:] = x[b, lo: base + QTOK]
        xt_c = np.ascontiguousarray(
            xsh.T.reshape(ND, 128, TSH).transpose(1, 0, 2))
        xh_c, xl_c = _split8(xt_c)
        logb_c = logb.copy()
        if hf == 1:
            logb_c[:, :, 0, :] = logb_c[:, :, 1, :]
        in_maps.append({
            "xh": xh_c, "xl": xl_c,
            "wqh": wqh, "wql": wql, "wkh": wkh, "wkl": wkl,
            "wvh": wvh, "wvl": wvl, "woh": woh, "wol": wol,
            "invu2": invu2, "ident": ident_b,
            "logb": np.ascontiguousarray(logb_c.astype(ml_dtypes.bfloat16)),
        })

    return in_maps


def kernel(x, wq, wk, wv, wo, q_norm_w, k_norm_w):
    if "nc" not in _CACHE:
        _CACHE["nc"] = _build_program()
    nc = _CACHE["nc"]
    in_maps = _make_in_maps(x, wq, wk, wv, wo, q_norm_w, k_norm_w)
    _CACHE["in_maps"] = in_maps
    import time as _time
    last_err = None
    for attempt in range(3):
        try:
            res = run_bass_kernel_spmd(nc, in_maps, core_ids=list(range(8)))
            break
        except Exception as e:  # transient NRT/device wedges recover on retry
            last_err = e
            _time.sleep(10 * (attempt + 1))
    else:
        raise last_err

    out = np.empty((B, T, DIM), dtype=np.float32)
    for c in range(8):
        b, hf = c // 2, c % 2
        out[b, hf * QTOK:(hf + 1) * QTOK, :] = res.results[c]["out"]
    return out


# revision 7
# speedup vs baseline: 1.1547x; 1.1547x over previous
"""Trainium2 Bass kernel for nn_CodecAttention (sliding-window ALiBi attention).

Reference computation (B=4, T=2048, DIM=1024, H=8, HD=128, WINDOW=16):
    xq = rms_norm(x @ wq) ; xk = rms_norm(x @ wk) ; xv = x @ wv
    scores = q k^T / sqrt(HD) + alibi_bias  (causal + 16-token sliding window)
    out = softmax(scores) @ v  -> reshape -> @ wo

Sharding: 8 cores = (batch b, sequence half). Each core processes 1024 query
tokens plus a 16-token key/value halo (zeros for the first half), fully
locally -- no collectives.

v3 design notes:
  - All four projections run as fp8e4 DoubleRow matmuls with a 3-chain hi/lo
    decomposition (x_hi*w_hi + x_lo*w_hi + x_hi*w_lo): 12 DR instructions per
    1024-deep contraction instead of 8 bf16 ones (0.75x PE cost) at
    bf16-level accuracy.  Weight hi parts are pre-scaled per tensor (SC_*);
    the scale is folded into the RMS rsqrt (q/k) or drain scale (v/wo).
  - RMS norm: drain PSUM -> bf16, squares via one DVE stt per chunk, then a
    chained ones[128,128] matmul accumulates across the 8 feature tiles AND
    broadcasts the per-token sum to all partitions; ACT Sqrt (scale folds
    SC^2, bias = eps*SC^2) + DVE reciprocal + 8 per-chunk multiplies.  The
    reduce+apply of chunk group g is emitted after group g+1's projection
    matmuls so the PE never waits on the square tiles.
  - attention per (head, 128-query tile): ALiBi+mask bias (log domain,
    masked = -30000 -> exp gives exact 0) is preloaded into PSUM via a
    144-wide ident matmul, QK accumulates on top; ACT Exp drains PSUM->SBUF
    with accum_out producing the softmax denominator for free; DVE
    reciprocal + DVE/Pool per-partition normalize; the probs transpose runs
    on the DMA xbar (dma_start_transpose of a zero-padded [128,256] tile ->
    [128,2,128]); PV in bf16; attention output drained as fp8 hi (ACT) +
    lo (DVE stt) pair.  Each query tile runs as two passes (all QK/exp
    first, then all PV/drains) so no engine queue head-of-line-blocks on
    the DMA transpose latency; wo chains and v-projection tiles are
    interleaved between passes as PE fillers.
  - output projection: 3-chain fp8 DR over (aT_hi+aT_lo)@wo_hi + aT_hi@wo_lo,
    drained with the 1/SC_O scale folded in, DMA straight to DRAM.
"""

import math
import os

import numpy as np
import ml_dtypes

os.environ.setdefault("MYCRO_LOCAL_CACHE", "1")

import concourse.mybir as mybir
import concourse.tile as tile
from concourse import bacc
from concourse.bass_utils import run_bass_kernel_spmd

F32 = mybir.dt.float32
BF16 = mybir.dt.bfloat16
F8 = mybir.dt.float8e4
AF = mybir.ActivationFunctionType
ALU = mybir.AluOpType
DR = mybir.MatmulPerfMode.DoubleRow

B, T, DIM = 4, 2048, 1024
H, HD = 8, 128
WINDOW = 16
EPS = 1e-6

HALO = 16                  # key/value halo tokens per shard
TSH = HALO + T // 2        # 1040 k/v tokens per shard
QTOK = T // 2              # 1024 query tokens per shard
ND = DIM // 128            # 8 dim tiles
NVT = 9                    # v token tiles (8*128 + 16)
NQT = QTOK // 128          # 8 query tiles
KW = 128 + HALO            # 144 keys per query tile

SC_Q = 256.0               # fp8 pre-scale for wq
SC_K = 2048.0              # fp8 pre-scale for wk*u (u ~ 1/11.3 folded in)
SC_V = 256.0               # fp8 pre-scale for wv
SC_O = 256.0               # fp8 pre-scale for wo

NEG_MASK = -30000.0        # log-domain mask; exp() underflows to exact 0

_SLOPES = [2.0 ** (-i) for i in range(H)]

_CACHE = {}


def _build_program():
    nc = bacc.Bacc("TRN2", debug=False, target_bir_lowering=False, num_devices=8)

    xh = nc.declare_dram_parameter("xh", [128, ND, TSH], F8, isOutput=False)
    xl = nc.declare_dram_parameter("xl", [128, ND, TSH], F8, isOutput=False)
    w_in = {}
    for wn in ("wq", "wk", "wv", "wo"):
        for p in ("h", "l"):
            w_in[wn + p] = nc.declare_dram_parameter(
                wn + p, [128, ND, DIM], F8, isOutput=False)
    invu2 = nc.declare_dram_parameter("invu2", [128, ND], F32, isOutput=False)
    ident = nc.declare_dram_parameter("ident", [128, 128], BF16, isOutput=False)
    logb = nc.declare_dram_parameter("logb", [128, H, 2, KW], BF16,
                                     isOutput=False)
    out = nc.declare_dram_parameter("out", [QTOK, DIM], F32, isOutput=True)

    with tile.TileContext(nc) as tc:
        _emit(tc, nc, xh, xl, w_in, invu2, ident, logb, out)
    nc.compile()
    return nc


def _emit(tc, nc, xh, xl, w_in, invu2, ident, logb, out):
    with (
        tc.tile_pool(name="big", bufs=1) as big,
        tc.tile_pool(name="sq", bufs=int(os.environ.get("KP_SQ", 18))) as sqp,
        tc.tile_pool(name="rst", bufs=4) as rstp,
        tc.tile_pool(name="ex", bufs=int(os.environ.get("KP_EX", 9))) as exp_,
        tc.tile_pool(name="exn", bufs=int(os.environ.get("KP_EXN", 9))) as exnp,
        tc.tile_pool(name="ext", bufs=int(os.environ.get("KP_EXT", 9))) as extp,
        tc.tile_pool(name="rs", bufs=10) as rsp,
        tc.tile_pool(name="at", bufs=int(os.environ.get("KP_AT", 3))) as atp,
        tc.tile_pool(name="ob", bufs=3) as obp,
        tc.tile_pool(name="pp", bufs=int(os.environ.get("KP_PP", 3)),
                     space="PSUM") as pp,
        tc.tile_pool(name="sq2", bufs=int(os.environ.get("KP_SQ2", 3)),
                     space="PSUM") as sqp2,
        tc.tile_pool(name="yt", bufs=int(os.environ.get("KP_YT", 2)),
                     space="PSUM") as ytp,
    ):
        kt_sb = big.tile([128, ND, TSH], BF16)
        qt_sb = big.tile([128, ND, QTOK], BF16)
        v_sb = big.tile([128, NVT, DIM], BF16)
        xh_sb = big.tile([128, ND, TSH], F8)
        xl_sb = big.tile([128, ND, TSH], F8)
        ws = {}
        for wn in ("wq", "wk", "wv", "wo"):
            for p in ("h", "l"):
                ws[wn + p] = big.tile([128, ND, DIM], F8, name=wn + p)
        invu2_sb = big.tile([128, ND], F32)
        ident_sb = big.tile([128, 128], BF16)
        logb_sb = big.tile([128, H, 2, KW], BF16)
        ones_sb = big.tile([128, 128], BF16)
        epsk_sb = big.tile([128, 1], F32)
        epsq_sb = big.tile([128, 1], F32)
        nc.vector.memset(ones_sb[:], 1.0)
        nc.vector.memset(epsk_sb[:], EPS * SC_K * SC_K)
        nc.vector.memset(epsq_sb[:], EPS * SC_Q * SC_Q)

        # ---- input DMAs: k weights + x first (kk-pair granularity), then
        # the rest as whole-tensor transfers.
        for j in range(ND // 2):
            nc.sync.dma_start(ws["wkh"][:, 2 * j:2 * j + 2, :],
                              w_in["wkh"][:, 2 * j:2 * j + 2, :])
            nc.sync.dma_start(xh_sb[:, 2 * j:2 * j + 2, :],
                              xh[:, 2 * j:2 * j + 2, :])
            nc.sync.dma_start(ws["wkl"][:, 2 * j:2 * j + 2, :],
                              w_in["wkl"][:, 2 * j:2 * j + 2, :])
            nc.sync.dma_start(xl_sb[:, 2 * j:2 * j + 2, :],
                              xl[:, 2 * j:2 * j + 2, :])
        nc.sync.dma_start(invu2_sb[:], invu2[:])
        nc.sync.dma_start(ident_sb[:], ident[:])
        for wn in ("wqh", "wql", "wvh", "wvl", "woh", "wol"):
            nc.sync.dma_start(ws[wn][:], w_in[wn][:])
        nc.sync.dma_start(logb_sb[:], logb[:])

        # ---------------- projection helpers ----------------
        def proj_chains(ps, wh_sb, wl_sb, m, c0, cw):
            """12 DR matmuls: xh*wh + xl*wh + xh*wl accumulated in psum."""
            for ci, (mov, sta) in enumerate(
                    [(xh_sb, wh_sb), (xl_sb, wh_sb), (xh_sb, wl_sb)]):
                for j in range(ND // 2):
                    nc.tensor.matmul(
                        ps[:, :cw],
                        sta[:, 2 * j:2 * j + 2, m * 128:(m + 1) * 128],
                        mov[:, 2 * j:2 * j + 2, c0:c0 + cw],
                        start=(ci == 0 and j == 0),
                        stop=(ci == 2 and j == ND // 2 - 1),
                        perf_mode=DR)

        def proj_group(dst, wh_sb, wl_sb, src_c0, dst_c0, cw, sq_scal, di):
            """Project one token chunk; returns square tiles for the rms."""
            sq_list = []
            for m in range(ND):
                ps = pp.tile([128, 512], F32, tag="ps", name="ps")
                proj_chains(ps, wh_sb, wl_sb, m, src_c0, cw)
                d = dst[:, m, dst_c0:dst_c0 + cw]
                if (di + m) % 2 == 0:
                    nc.scalar.copy(d, ps[:, :cw])
                else:
                    nc.vector.tensor_copy(d, ps[:, :cw])
                sq = sqp.tile([128, 512], BF16, tag="sq", name="sq")
                scal = sq_scal if isinstance(sq_scal, float) \
                    else sq_scal[:, m:m + 1]
                nc.vector.scalar_tensor_tensor(
                    out=sq[:, :cw], in0=d, scalar=scal,
                    in1=d, op0=ALU.mult, op1=ALU.mult)
                sq_list.append(sq)
            return sq_list

        def rms_reduce(dst, dst_c0, cw, sq_list, eps_sb, sc2):
            """ones-matmul partition reduce+broadcast, rsqrt, apply."""
            ssb = pp.tile([128, 512], F32, tag="ps", name="ssb")
            for m in range(ND):
                nc.tensor.matmul(ssb[:, :cw], ones_sb[:], sq_list[m][:, :cw],
                                 start=(m == 0), stop=(m == ND - 1))
            rms = rstp.tile([128, 512], F32, tag="rms", name="rms")
            nc.scalar.activation(rms[:, :cw], ssb[:, :cw], AF.Sqrt,
                                 bias=eps_sb[:], scale=sc2)
            rstd = rstp.tile([128, 512], BF16, tag="rstd", name="rstd")
            with nc.allow_low_precision(reason="bf16 rstd"):
                nc.vector.reciprocal(rstd[:, :cw], rms[:, :cw])
            for m in range(ND):
                eng = nc.gpsimd if m % 2 == 0 else nc.vector
                eng.tensor_mul(dst[:, m, dst_c0:dst_c0 + cw],
                               dst[:, m, dst_c0:dst_c0 + cw], rstd[:, :cw])

        def v_tile(tt):
            tw = 128 if tt < 8 else 16
            for nn in range(2):
                ps = pp.tile([128, 512], F32, tag="ps", name="ps")
                for ci, (sta, mov) in enumerate(
                        [(xh_sb, ws["wvh"]), (xl_sb, ws["wvh"]),
                         (xh_sb, ws["wvl"])]):
                    for j in range(ND // 2):
                        nc.tensor.matmul(
                            ps[:tw, :],
                            sta[:, 2 * j:2 * j + 2, tt * 128:tt * 128 + tw],
                            mov[:, 2 * j:2 * j + 2, nn * 512:(nn + 1) * 512],
                            start=(ci == 0 and j == 0),
                            stop=(ci == 2 and j == ND // 2 - 1),
                            perf_mode=DR)
                dstv = v_sb[:tw, tt, nn * 512:(nn + 1) * 512]
                if (tt + nn) % 2 == 0:
                    nc.scalar.activation(dstv, ps[:tw, :], AF.Copy,
                                         scale=1.0 / SC_V)
                else:
                    nc.vector.tensor_scalar_mul(dstv, ps[:tw, :], 1.0 / SC_V)

        def wo_proj(t, aTh, aTl, nn):
            ps_o = pp.tile([128, 512], F32, tag="ps", name="ps_o")
            for ci, (sta, mov) in enumerate(
                    [(aTh, ws["woh"]), (aTl, ws["woh"]), (aTh, ws["wol"])]):
                for j in range(ND // 2):
                    nc.tensor.matmul(
                        ps_o[:],
                        sta[:, 2 * j:2 * j + 2, :],
                        mov[:, 2 * j:2 * j + 2, nn * 512:(nn + 1) * 512],
                        start=(ci == 0 and j == 0),
                        stop=(ci == 2 and j == ND // 2 - 1),
                        perf_mode=DR)
            o_sb = obp.tile([128, 512], F32, tag="osb", name="o_sb")
            if nn % 2 == 0:
                nc.vector.tensor_scalar_mul(o_sb[:], ps_o[:], 1.0 / SC_O)
            else:
                nc.scalar.activation(o_sb[:], ps_o[:], AF.Copy,
                                     scale=1.0 / SC_O)
            nc.sync.dma_start(
                out[t * 128:(t + 1) * 128, nn * 512:(nn + 1) * 512],
                o_sb[:])

        # ---------------- attention tile (two passes) ----------------
        NEXN = int(os.environ.get("KP_EXN", 9))
        exn_init = [0]

        def attn_passA(t, fillers):
            """QK + exp + normalize + transpose for all 8 heads of tile t.
            `fillers` are PE-filler closures emitted between QK pairs."""
            var = 0 if t == 0 else 1
            exsTs = []
            fi = 0
            for h in range(H):
                sQ_t = sqp2.tile([128, KW], F32, tag="sq", name="sQ")
                sQ = sQ_t[:, :]
                nc.tensor.matmul(sQ, ident_sb[:], logb_sb[:, h, var, :],
                                 start=True, stop=False)
                nc.tensor.matmul(sQ,
                                 qt_sb[:, h, t * 128:(t + 1) * 128],
                                 kt_sb[:, h, t * 128:t * 128 + KW],
                                 start=False, stop=True)
                ex = exp_.tile([128, KW], BF16, tag="ex", name="ex")
                rs = rsp.tile([128, 1], F32, tag="rs", name="rs")
                nc.scalar.activation(ex[:], sQ, AF.Exp, accum_out=rs[:])
                rcp = rsp.tile([128, 1], F32, tag="rcp", name="rcp")
                nc.vector.reciprocal(rcp[:], rs[:])
                exn = exnp.tile([128, 256], BF16, tag="exn", name="exn")
                if exn_init[0] < NEXN:
                    nc.vector.memset(exn[:, KW:256], 0.0)
                    exn_init[0] += 1
                if h % 2 == 0:
                    nc.gpsimd.tensor_scalar_mul(exn[:, 0:KW], ex[:], rcp[:])
                else:
                    nc.vector.tensor_scalar_mul(exn[:, 0:KW], ex[:], rcp[:])
                exsT = extp.tile([128, 2, 128], BF16, tag="exsT", name="exsT")
                nc.sync.dma_start_transpose(exsT[:], exn[:])
                exsTs.append(exsT)
                if h in (2, 5) and fi < len(fillers):
                    fillers[fi]()
                    fi += 1
            while fi < len(fillers):
                fillers[fi]()
                fi += 1
            return exsTs

        def attn_passB(t, exsTs, aTh, aTl):
            for h in range(H):
                exsT = exsTs[h]
                yT_t = ytp.tile([128, 128], F32, tag="yT", name="yT")
                yT = yT_t[:, :]
                hs = slice(h * 128, (h + 1) * 128)
                nc.tensor.matmul(yT, v_sb[:, t, hs], exsT[:, 0, :],
                                 start=True, stop=False)
                nc.tensor.matmul(yT, v_sb[0:16, t + 1, hs],
                                 exsT[0:16, 1, :], start=False, stop=True)
                with nc.allow_low_precision(reason="fp8 hi/lo attention out"):
                    nc.scalar.copy(aTh[:, h, :], yT)
                    nc.vector.scalar_tensor_tensor(
                        out=aTl[:, h, :], in0=yT, scalar=1.0,
                        in1=aTh[:, h, :], op0=ALU.mult, op1=ALU.subtract)

        # ---------------- emission schedule ----------------
        cq = 1.0 / (SC_Q * SC_Q * DIM)
        sq_k0 = proj_group(kt_sb, ws["wkh"], ws["wkl"], 0, 0, 512, invu2_sb, 0)
        sq_k1 = proj_group(kt_sb, ws["wkh"], ws["wkl"], 512, 512, 512,
                           invu2_sb, 1)
        rms_reduce(kt_sb, 0, 512, sq_k0, epsk_sb, SC_K * SC_K)
        sq_kt = proj_group(kt_sb, ws["wkh"], ws["wkl"], 1024, 1024, 16,
                           invu2_sb, 2)
        rms_reduce(kt_sb, 512, 512, sq_k1, epsk_sb, SC_K * SC_K)
        sq_q0 = proj_group(qt_sb, ws["wqh"], ws["wql"], HALO, 0, 512, cq, 0)
        rms_reduce(kt_sb, 1024, 16, sq_kt, epsk_sb, SC_K * SC_K)
        sq_q1 = proj_group(qt_sb, ws["wqh"], ws["wql"], HALO + 512, 512, 512,
                           cq, 1)
        rms_reduce(qt_sb, 0, 512, sq_q0, epsq_sb, SC_Q * SC_Q)
        v_tile(0)
        rms_reduce(qt_sb, 512, 512, sq_q1, epsq_sb, SC_Q * SC_Q)
        v_tile(1)
        v_tile(2)
        v_tile(3)

        prev = None
        for t in range(NQT):
            aTh = atp.tile([128, ND, 128], F8, tag="aTh", name="aTh")
            aTl = atp.tile([128, ND, 128], F8, tag="aTl", name="aTl")
            fillers = []
            if prev is not None:
                fillers.append(lambda t=t, p=prev: wo_proj(t - 1, p[0], p[1], 0))
                fillers.append(lambda t=t, p=prev: wo_proj(t - 1, p[0], p[1], 1))
            if t + 4 < NVT:
                fillers.append(lambda tt=t + 4: v_tile(tt))
            exsTs = attn_passA(t, fillers)
            attn_passB(t, exsTs, aTh, aTl)
            prev = (aTh, aTl)
        wo_proj(NQT - 1, prev[0], prev[1], 0)
        wo_proj(NQT - 1, prev[0], prev[1], 1)


def _host_constants():
    # logb[i, c] = slope * (c - i - 16) inside the band (-16 <= c-i-16 <= 0),
    # else NEG_MASK.  Variant 0 additionally masks kt cols < 16 (halo before
    # sequence start).
    ii = np.arange(128)[:, None]
    cc = np.arange(KW)[None, :]
    rel = cc - ii - HALO
    band = (rel <= 0) & (rel >= -WINDOW)
    logb = np.full((128, H, 2, KW), NEG_MASK, dtype=np.float32)
    for h in range(H):
        pat = np.where(band, _SLOPES[h] * rel, NEG_MASK)
        logb[:, h, 1, :] = pat
        logb[:, h, 0, :] = np.where(cc < HALO, NEG_MASK, pat)
    ident = np.eye(128, dtype=np.float32)
    return logb, ident


def _split8(a):
    """fp8 hi/lo split of a float32 array."""
    f8 = ml_dtypes.float8_e4m3
    hi = a.astype(f8)
    lo = (a - hi.astype(np.float32)).astype(f8)
    return hi, lo


def _wlayout(w):
    """[DIM, DIM] -> [128, ND, DIM] (partition = row % 128, dim1 = row // 128)."""
    return np.ascontiguousarray(
        w.reshape(ND, 128, w.shape[1]).transpose(1, 0, 2))


def _make_in_maps(x, wq, wk, wv, wo, q_norm_w, k_norm_w):
    x = np.asarray(x, dtype=np.float32)
    wq = np.asarray(wq, dtype=np.float32)
    wk = np.asarray(wk, dtype=np.float32)
    wv = np.asarray(wv, dtype=np.float32)
    wo = np.asarray(wo, dtype=np.float32)
    q_norm_w = np.asarray(q_norm_w, dtype=np.float32)
    k_norm_w = np.asarray(k_norm_w, dtype=np.float32)

    u = (q_norm_w * k_norm_w / math.sqrt(HD)).astype(np.float32)
    wqh, wql = _split8(_wlayout(wq * SC_Q))
    wkh, wkl = _split8(_wlayout(wk * u[None, :] * SC_K))
    wvh, wvl = _split8(_wlayout(wv * SC_V))
    woh, wol = _split8(_wlayout(wo * SC_O))
    # raw sum-of-squares correction: mean_f k_raw^2 = sum_f k''^2 * invu2
    invu2 = np.ascontiguousarray(
        (1.0 / (u * u * SC_K * SC_K * DIM)).reshape(ND, 128).T
        .astype(np.float32))

    logb, ident = _host_constants()
    ident_b = ident.astype(ml_dtypes.bfloat16)

    in_maps = []
    for c in range(8):
        b, hf = c // 2, c % 2
        base = hf * (T // 2)
        xsh = np.zeros((TSH, DIM), dtype=np.float32)
        lo = base - HALO
        if lo < 0:
            xsh[HALO:] = x[b, base: base + QTOK]
        else:
            xsh[:] = x[b, lo: base + QTOK]
        xt_c = np.ascontiguousarray(
            xsh.T.reshape(ND, 128, TSH).transpose(1, 0, 2))
        xh_c, xl_c = _split8(xt_c)
        logb_c = logb.copy()
        if hf == 1:
            logb_c[:, :, 0, :] = logb_c[:, :, 1, :]
        in_maps.append({
            "xh": xh_c, "xl": xl_c,
            "wqh": wqh, "wql": wql, "wkh": wkh, "wkl": wkl,
            "wvh": wvh, "wvl": wvl, "woh": woh, "wol": wol,
            "invu2": invu2, "ident": ident_b,
            "logb": np.ascontiguousarray(logb_c.astype(ml_dtypes.bfloat16)),
        })

    return in_maps


def kernel(x, wq, wk, wv, wo, q_norm_w, k_norm_w):
    if "nc" not in _CACHE:
        _CACHE["nc"] = _build_program()
    nc = _CACHE["nc"]
    in_maps = _make_in_maps(x, wq, wk, wv, wo, q_norm_w, k_norm_w)
    _CACHE["in_maps"] = in_maps
    import time as _time
    last_err = None
    for attempt in range(3):
        try:
            res = run_bass_kernel_spmd(nc, in_maps, core_ids=list(range(8)))
            break
        except Exception as e:  # transient NRT/device wedges recover on retry
            last_err = e
            _time.sleep(10 * (attempt + 1))
    else:
        raise last_err

    out = np.empty((B, T, DIM), dtype=np.float32)
    for c in range(8):
        b, hf = c // 2, c % 2
        out[b, hf * QTOK:(hf + 1) * QTOK, :] = res.results[c]["out"]
    return out


# revision 11
# speedup vs baseline: 1.2464x; 1.0795x over previous
"""Trainium2 Bass kernel for nn_CodecAttention (sliding-window ALiBi attention).

Reference computation (B=4, T=2048, DIM=1024, H=8, HD=128, WINDOW=16):
    xq = rms_norm(x @ wq) ; xk = rms_norm(x @ wk) ; xv = x @ wv
    scores = q k^T / sqrt(HD) + alibi_bias  (causal + 16-token sliding window)
    out = softmax(scores) @ v  -> reshape -> @ wo

Sharding: 8 cores = (batch b, sequence half). Each core processes 1024 query
tokens plus a 16-token key/value halo (zeros for the first half), fully
locally -- no collectives.

v3 design notes:
  - All four projections run as fp8e4 DoubleRow matmuls with a 3-chain hi/lo
    decomposition (x_hi*w_hi + x_lo*w_hi + x_hi*w_lo): 12 DR instructions per
    1024-deep contraction instead of 8 bf16 ones (0.75x PE cost) at
    bf16-level accuracy.  Weight hi parts are pre-scaled per tensor (SC_*);
    the scale is folded into the RMS rsqrt (q/k) or drain scale (v/wo).
  - RMS norm: drain PSUM -> bf16, squares via one DVE stt per chunk, then a
    chained ones[128,128] matmul accumulates across the 8 feature tiles AND
    broadcasts the per-token sum to all partitions; ACT Sqrt (scale folds
    SC^2, bias = eps*SC^2) + DVE reciprocal + 8 per-chunk multiplies.  The
    reduce+apply of chunk group g is emitted after group g+1's projection
    matmuls so the PE never waits on the square tiles.
  - attention per (head, 128-query tile): ALiBi+mask bias (log domain,
    masked = -30000 -> exp gives exact 0) is preloaded into PSUM via a
    144-wide ident matmul, QK accumulates on top; ACT Exp drains PSUM->SBUF
    with accum_out producing the softmax denominator for free; DVE
    reciprocal + DVE/Pool per-partition normalize; the probs transpose runs
    on the DMA xbar (dma_start_transpose of a zero-padded [128,256] tile ->
    [128,2,128]); PV in bf16; attention output drained as fp8 hi (ACT) +
    lo (DVE stt) pair.  Each query tile runs as two passes (all QK/exp
    first, then all PV/drains) so no engine queue head-of-line-blocks on
    the DMA transpose latency; wo chains and v-projection tiles are
    interleaved between passes as PE fillers.
  - output projection: 3-chain fp8 DR over (aT_hi+aT_lo)@wo_hi + aT_hi@wo_lo,
    drained with the 1/SC_O scale folded in, DMA straight to DRAM.
"""

import math
import os

import numpy as np
import ml_dtypes

os.environ.setdefault("MYCRO_LOCAL_CACHE", "1")

import concourse.mybir as mybir
import concourse.tile as tile
from concourse import bacc
from concourse.bass_utils import run_bass_kernel_spmd

F32 = mybir.dt.float32
BF16 = mybir.dt.bfloat16
F8 = mybir.dt.float8e4
AF = mybir.ActivationFunctionType
ALU = mybir.AluOpType
DR = mybir.MatmulPerfMode.DoubleRow

B, T, DIM = 4, 2048, 1024
H, HD = 8, 128
WINDOW = 16
EPS = 1e-6

HALO = 16                  # key/value halo tokens per shard
TSH = HALO + T // 2        # 1040 k/v tokens per shard
QTOK = T // 2              # 1024 query tokens per shard
ND = DIM // 128            # 8 dim tiles
NVT = 9                    # v token tiles (8*128 + 16)
NQT = QTOK // 128          # 8 query tiles
KW = 128 + HALO            # 144 keys per query tile

SC_Q = 256.0               # fp8 pre-scale for wq
SC_K = 2048.0              # fp8 pre-scale for wk*u (u ~ 1/11.3 folded in)
SC_V = 256.0               # fp8 pre-scale for wv
SC_O = 256.0               # fp8 pre-scale for wo

NEG_MASK = -30000.0        # log-domain mask; exp() underflows to exact 0

_SLOPES = [2.0 ** (-i) for i in range(H)]

_CACHE = {}


def _build_program():
    nc = bacc.Bacc("TRN2", debug=False, target_bir_lowering=False, num_devices=8)

    xh = nc.declare_dram_parameter("xh", [128, ND, TSH], F8, isOutput=False)
    xl = nc.declare_dram_parameter("xl", [128, ND, TSH], F8, isOutput=False)
    w_in = {}
    for wn in ("wq", "wk", "wv", "wo"):
        for p in ("h", "l"):
            w_in[wn + p] = nc.declare_dram_parameter(
                wn + p, [128, ND, DIM], F8, isOutput=False)
    invu2 = nc.declare_dram_parameter("invu2", [128, ND], F32, isOutput=False)
    ident = nc.declare_dram_parameter("ident", [128, 128], BF16, isOutput=False)
    logb = nc.declare_dram_parameter("logb", [128, H, 2, KW], BF16,
                                     isOutput=False)
    out = nc.declare_dram_parameter("out", [QTOK, DIM], F32, isOutput=True)

    with tile.TileContext(nc) as tc:
        _emit(tc, nc, xh, xl, w_in, invu2, ident, logb, out)
    nc.compile()
    return nc


def _emit(tc, nc, xh, xl, w_in, invu2, ident, logb, out):
    with (
        tc.tile_pool(name="big", bufs=1) as big,
        tc.tile_pool(name="sq", bufs=int(os.environ.get("KP_SQ", 18))) as sqp,
        tc.tile_pool(name="rst", bufs=4) as rstp,
        tc.tile_pool(name="ex", bufs=int(os.environ.get("KP_EX", 9))) as exp_,
        tc.tile_pool(name="exn", bufs=int(os.environ.get("KP_EXN", 9))) as exnp,
        tc.tile_pool(name="ext", bufs=int(os.environ.get("KP_EXT", 17))) as extp,
        tc.tile_pool(name="rs", bufs=10) as rsp,
        tc.tile_pool(name="at", bufs=int(os.environ.get("KP_AT", 3))) as atp,
        tc.tile_pool(name="ob", bufs=3) as obp,
        tc.tile_pool(name="pp", bufs=int(os.environ.get("KP_PP", 3)),
                     space="PSUM") as pp,
        tc.tile_pool(name="sq2", bufs=int(os.environ.get("KP_SQ2", 3)),
                     space="PSUM") as sqp2,
        tc.tile_pool(name="yt", bufs=int(os.environ.get("KP_YT", 2)),
                     space="PSUM") as ytp,
    ):
        kt_sb = big.tile([128, ND, TSH], BF16)
        qt_sb = big.tile([128, ND, QTOK], BF16)
        v_sb = big.tile([128, NVT, DIM], BF16)
        xh_sb = big.tile([128, ND, TSH], F8)
        xl_sb = big.tile([128, ND, TSH], F8)
        ws = {}
        for wn in ("wq", "wk", "wv", "wo"):
            for p in ("h", "l"):
                ws[wn + p] = big.tile([128, ND, DIM], F8, name=wn + p)
        invu2_sb = big.tile([128, ND], F32)
        ident_sb = big.tile([128, 128], BF16)
        logb_sb = big.tile([128, H, 2, KW], BF16)
        ones_sb = big.tile([128, 128], BF16)
        epsk_sb = big.tile([128, 1], F32)
        epsq_sb = big.tile([128, 1], F32)
        nc.vector.memset(ones_sb[:], 1.0)
        nc.vector.memset(epsk_sb[:], EPS * SC_K * SC_K)
        nc.vector.memset(epsq_sb[:], EPS * SC_Q * SC_Q)

        # ---- input DMAs: k weights + x first (kk-pair granularity), then
        # the rest as whole-tensor transfers.
        for j in range(ND // 2):
            nc.sync.dma_start(ws["wkh"][:, 2 * j:2 * j + 2, :],
                              w_in["wkh"][:, 2 * j:2 * j + 2, :])
            nc.sync.dma_start(xh_sb[:, 2 * j:2 * j + 2, :],
                              xh[:, 2 * j:2 * j + 2, :])
            nc.sync.dma_start(ws["wkl"][:, 2 * j:2 * j + 2, :],
                              w_in["wkl"][:, 2 * j:2 * j + 2, :])
            nc.sync.dma_start(xl_sb[:, 2 * j:2 * j + 2, :],
                              xl[:, 2 * j:2 * j + 2, :])
        nc.sync.dma_start(invu2_sb[:], invu2[:])
        nc.sync.dma_start(ident_sb[:], ident[:])
        for wn in ("wqh", "wql", "wvh", "wvl", "woh", "wol"):
            nc.sync.dma_start(ws[wn][:], w_in[wn][:])
        nc.sync.dma_start(logb_sb[:], logb[:])

        # ---------------- projection helpers ----------------
        def proj_chains(ps, wh_sb, wl_sb, m, c0, cw):
            """12 DR matmuls: xh*wh + xl*wh + xh*wl accumulated in psum."""
            for ci, (mov, sta) in enumerate(
                    [(xh_sb, wh_sb), (xl_sb, wh_sb), (xh_sb, wl_sb)]):
                for j in range(ND // 2):
                    nc.tensor.matmul(
                        ps[:, :cw],
                        sta[:, 2 * j:2 * j + 2, m * 128:(m + 1) * 128],
                        mov[:, 2 * j:2 * j + 2, c0:c0 + cw],
                        start=(ci == 0 and j == 0),
                        stop=(ci == 2 and j == ND // 2 - 1),
                        perf_mode=DR)

        def proj_group(dst, wh_sb, wl_sb, src_c0, dst_c0, cw, sq_scal, di):
            """Project one token chunk; returns square tiles for the rms."""
            sq_list = []
            for m in range(ND):
                ps = pp.tile([128, 512], F32, tag="ps", name="ps")
                proj_chains(ps, wh_sb, wl_sb, m, src_c0, cw)
                d = dst[:, m, dst_c0:dst_c0 + cw]
                if (di + m) % 2 == 0:
                    nc.scalar.copy(d, ps[:, :cw])
                else:
                    nc.vector.tensor_copy(d, ps[:, :cw])
                sq = sqp.tile([128, 512], BF16, tag="sq", name="sq")
                scal = sq_scal if isinstance(sq_scal, float) \
                    else sq_scal[:, m:m + 1]
                nc.vector.scalar_tensor_tensor(
                    out=sq[:, :cw], in0=d, scalar=scal,
                    in1=d, op0=ALU.mult, op1=ALU.mult)
                sq_list.append(sq)
            return sq_list

        def rms_reduce(dst, dst_c0, cw, sq_list, eps_sb, sc2):
            """ones-matmul partition reduce+broadcast, rsqrt, apply."""
            ssb = pp.tile([128, 512], F32, tag="ps", name="ssb")
            for m in range(ND):
                nc.tensor.matmul(ssb[:, :cw], ones_sb[:], sq_list[m][:, :cw],
                                 start=(m == 0), stop=(m == ND - 1))
            rms = rstp.tile([128, 512], F32, tag="rms", name="rms")
            nc.scalar.activation(rms[:, :cw], ssb[:, :cw], AF.Sqrt,
                                 bias=eps_sb[:], scale=sc2)
            rstd = rstp.tile([128, 512], BF16, tag="rstd", name="rstd")
            with nc.allow_low_precision(reason="bf16 rstd"):
                nc.vector.reciprocal(rstd[:, :cw], rms[:, :cw])
            for m in range(ND):
                eng = nc.gpsimd if m % 2 == 0 else nc.vector
                eng.tensor_mul(dst[:, m, dst_c0:dst_c0 + cw],
                               dst[:, m, dst_c0:dst_c0 + cw], rstd[:, :cw])

        def v_tile(tt):
            tw = 128 if tt < 8 else 16
            for nn in range(2):
                ps = pp.tile([128, 512], F32, tag="ps", name="ps")
                for ci, (sta, mov) in enumerate(
                        [(xh_sb, ws["wvh"]), (xl_sb, ws["wvh"]),
                         (xh_sb, ws["wvl"])]):
                    for j in range(ND // 2):
                        nc.tensor.matmul(
                            ps[:tw, :],
                            sta[:, 2 * j:2 * j + 2, tt * 128:tt * 128 + tw],
                            mov[:, 2 * j:2 * j + 2, nn * 512:(nn + 1) * 512],
                            start=(ci == 0 and j == 0),
                            stop=(ci == 2 and j == ND // 2 - 1),
                            perf_mode=DR)
                dstv = v_sb[:tw, tt, nn * 512:(nn + 1) * 512]
                if (tt + nn) % 2 == 0:
                    nc.scalar.activation(dstv, ps[:tw, :], AF.Copy,
                                         scale=1.0 / SC_V)
                else:
                    nc.vector.tensor_scalar_mul(dstv, ps[:tw, :], 1.0 / SC_V)

        def wo_proj(t, aTh, aTl, nn):
            ps_o = pp.tile([128, 512], F32, tag="ps", name="ps_o")
            for ci, (sta, mov) in enumerate(
                    [(aTh, ws["woh"]), (aTl, ws["woh"]), (aTh, ws["wol"])]):
                for j in range(ND // 2):
                    nc.tensor.matmul(
                        ps_o[:],
                        sta[:, 2 * j:2 * j + 2, :],
                        mov[:, 2 * j:2 * j + 2, nn * 512:(nn + 1) * 512],
                        start=(ci == 0 and j == 0),
                        stop=(ci == 2 and j == ND // 2 - 1),
                        perf_mode=DR)
            o_sb = obp.tile([128, 512], F32, tag="osb", name="o_sb")
            if nn % 2 == 0:
                nc.vector.tensor_scalar_mul(o_sb[:], ps_o[:], 1.0 / SC_O)
            else:
                nc.scalar.activation(o_sb[:], ps_o[:], AF.Copy,
                                     scale=1.0 / SC_O)
            nc.sync.dma_start(
                out[t * 128:(t + 1) * 128, nn * 512:(nn + 1) * 512],
                o_sb[:])

        # ---------------- attention tile (two passes) ----------------
        NEXN = int(os.environ.get("KP_EXN", 9))
        exn_init = [0]

        def attn_passA(t, fillers):
            """QK + exp + normalize + transpose for all 8 heads of tile t.
            `fillers` are PE-filler closures emitted between QK pairs."""
            var = 0 if t == 0 else 1
            exsTs = []
            fi = 0
            for h in range(H):
                sQ_t = sqp2.tile([128, KW], F32, tag="sq", name="sQ")
                sQ = sQ_t[:, :]
                nc.tensor.matmul(sQ, ident_sb[:], logb_sb[:, h, var, :],
                                 start=True, stop=False)
                nc.tensor.matmul(sQ,
                                 qt_sb[:, h, t * 128:(t + 1) * 128],
                                 kt_sb[:, h, t * 128:t * 128 + KW],
                                 start=False, stop=True)
                ex = exp_.tile([128, KW], BF16, tag="ex", name="ex")
                rs = rsp.tile([128, 1], F32, tag="rs", name="rs")
                nc.scalar.activation(ex[:], sQ, AF.Exp, accum_out=rs[:])
                rcp = rsp.tile([128, 1], F32, tag="rcp", name="rcp")
                nc.vector.reciprocal(rcp[:], rs[:])
                exn = exnp.tile([128, 256], BF16, tag="exn", name="exn")
                if exn_init[0] < NEXN:
                    nc.vector.memset(exn[:, KW:256], 0.0)
                    exn_init[0] += 1
                nc.gpsimd.tensor_scalar_mul(exn[:, 0:KW], ex[:], rcp[:])
                exsT = extp.tile([128, 2, 128], BF16, tag="exsT", name="exsT")
                nc.sync.dma_start_transpose(exsT[:], exn[:])
                exsTs.append(exsT)
                if h in (2, 5) and fi < len(fillers):
                    fillers[fi]()
                    fi += 1
            while fi < len(fillers):
                fillers[fi]()
                fi += 1
            return exsTs

        def attn_passB(t, exsTs, aTh, aTl, fillers=()):
            fi = 0
            for h in range(H):
                exsT = exsTs[h]
                yT_t = ytp.tile([128, 128], F32, tag="yT", name="yT")
                yT = yT_t[:, :]
                hs = slice(h * 128, (h + 1) * 128)
                nc.tensor.matmul(yT, v_sb[:, t, hs], exsT[:, 0, :],
                                 start=True, stop=False)
                nc.tensor.matmul(yT, v_sb[0:16, t + 1, hs],
                                 exsT[0:16, 1, :], start=False, stop=True)
                with nc.allow_low_precision(reason="fp8 hi/lo attention out"):
                    if h % 2 == 0:
                        nc.scalar.copy(aTh[:, h, :], yT)
                    else:
                        nc.vector.tensor_copy(aTh[:, h, :], yT)
                    nc.vector.scalar_tensor_tensor(
                        out=aTl[:, h, :], in0=yT, scalar=1.0,
                        in1=aTh[:, h, :], op0=ALU.mult, op1=ALU.subtract)
                if h == 3 and fi < len(fillers):
                    fillers[fi]()
                    fi += 1
            while fi < len(fillers):
                fillers[fi]()
                fi += 1

        # ---------------- emission schedule ----------------
        cq = 1.0 / (SC_Q * SC_Q * DIM)
        sq_k0 = proj_group(kt_sb, ws["wkh"], ws["wkl"], 0, 0, 512, invu2_sb, 0)
        sq_k1 = proj_group(kt_sb, ws["wkh"], ws["wkl"], 512, 512, 512,
                           invu2_sb, 1)
        rms_reduce(kt_sb, 0, 512, sq_k0, epsk_sb, SC_K * SC_K)
        sq_kt = proj_group(kt_sb, ws["wkh"], ws["wkl"], 1024, 1024, 16,
                           invu2_sb, 2)
        rms_reduce(kt_sb, 512, 512, sq_k1, epsk_sb, SC_K * SC_K)
        sq_q0 = proj_group(qt_sb, ws["wqh"], ws["wql"], HALO, 0, 512, cq, 0)
        rms_reduce(kt_sb, 1024, 16, sq_kt, epsk_sb, SC_K * SC_K)
        sq_q1 = proj_group(qt_sb, ws["wqh"], ws["wql"], HALO + 512, 512, 512,
                           cq, 1)
        rms_reduce(qt_sb, 0, 512, sq_q0, epsq_sb, SC_Q * SC_Q)
        v_tile(0)
        rms_reduce(qt_sb, 512, 512, sq_q1, epsq_sb, SC_Q * SC_Q)
        v_tile(1)
        v_tile(2)
        v_tile(3)

        # software pipeline: iteration t emits passA(t), then passB(t-1);
        # wo(t-2) chains and v tiles fill PE between the QK/PV bursts.
        pend = None      # (t-1): (aT tiles, exsTs) awaiting passB
        done = None      # (t-2): aT tiles awaiting wo projection
        for t in range(NQT):
            aTh = atp.tile([128, ND, 128], F8, tag="aTh", name="aTh")
            aTl = atp.tile([128, ND, 128], F8, tag="aTl", name="aTl")
            fillA, fillB = [], []
            if done is not None:
                fillA.append(lambda t=t, p=done: wo_proj(t - 2, p[0], p[1], 0))
                fillB.append(lambda t=t, p=done: wo_proj(t - 2, p[0], p[1], 1))
            if t + 4 < NVT:
                fillA.append(lambda tt=t + 4: v_tile(tt))
            exsTs = attn_passA(t, fillA)
            if pend is not None:
                attn_passB(t - 1, pend[1], pend[0][0], pend[0][1], fillB)
            done = pend[0] if pend is not None else None
            pend = ((aTh, aTl), exsTs)
        attn_passB(NQT - 1, pend[1], pend[0][0], pend[0][1])
        wo_proj(NQT - 2, done[0], done[1], 0)
        wo_proj(NQT - 2, done[0], done[1], 1)
        wo_proj(NQT - 1, pend[0][0], pend[0][1], 0)
        wo_proj(NQT - 1, pend[0][0], pend[0][1], 1)


def _host_constants():
    # logb[i, c] = slope * (c - i - 16) inside the band (-16 <= c-i-16 <= 0),
    # else NEG_MASK.  Variant 0 additionally masks kt cols < 16 (halo before
    # sequence start).
    ii = np.arange(128)[:, None]
    cc = np.arange(KW)[None, :]
    rel = cc - ii - HALO
    band = (rel <= 0) & (rel >= -WINDOW)
    logb = np.full((128, H, 2, KW), NEG_MASK, dtype=np.float32)
    for h in range(H):
        pat = np.where(band, _SLOPES[h] * rel, NEG_MASK)
        logb[:, h, 1, :] = pat
        logb[:, h, 0, :] = np.where(cc < HALO, NEG_MASK, pat)
    ident = np.eye(128, dtype=np.float32)
    return logb, ident


def _split8(a):
    """fp8 hi/lo split of a float32 array."""
    f8 = ml_dtypes.float8_e4m3
    hi = a.astype(f8)
    lo = (a - hi.astype(np.float32)).astype(f8)
    return hi, lo


def _wlayout(w):
    """[DIM, DIM] -> [128, ND, DIM] (partition = row % 128, dim1 = row // 128)."""
    return np.ascontiguousarray(
        w.reshape(ND, 128, w.shape[1]).transpose(1, 0, 2))


def _make_in_maps(x, wq, wk, wv, wo, q_norm_w, k_norm_w):
    x = np.asarray(x, dtype=np.float32)
    wq = np.asarray(wq, dtype=np.float32)
    wk = np.asarray(wk, dtype=np.float32)
    wv = np.asarray(wv, dtype=np.float32)
    wo = np.asarray(wo, dtype=np.float32)
    q_norm_w = np.asarray(q_norm_w, dtype=np.float32)
    k_norm_w = np.asarray(k_norm_w, dtype=np.float32)

    u = (q_norm_w * k_norm_w / math.sqrt(HD)).astype(np.float32)
    wqh, wql = _split8(_wlayout(wq * SC_Q))
    wkh, wkl = _split8(_wlayout(wk * u[None, :] * SC_K))
    wvh, wvl = _split8(_wlayout(wv * SC_V))
    woh, wol = _split8(_wlayout(wo * SC_O))
    # raw sum-of-squares correction: mean_f k_raw^2 = sum_f k''^2 * invu2
    invu2 = np.ascontiguousarray(
        (1.0 / (u * u * SC_K * SC_K * DIM)).reshape(ND, 128).T
        .astype(np.float32))

    logb, ident = _host_constants()
    ident_b = ident.astype(ml_dtypes.bfloat16)

    in_maps = []
    for c in range(8):
        b, hf = c // 2, c % 2
        base = hf * (T // 2)
        xsh = np.zeros((TSH, DIM), dtype=np.float32)
        lo = base - HALO
        if lo < 0:
            xsh[HALO:] = x[b, base: base + QTOK]
        else:
            xsh[:] = x[b, lo: base + QTOK]
        xt_c = np.ascontiguousarray(
            xsh.T.reshape(ND, 128, TSH).transpose(1, 0, 2))
        xh_c, xl_c = _split8(xt_c)
        logb_c = logb.copy()
        if hf == 1:
            logb_c[:, :, 0, :] = logb_c[:, :, 1, :]
        in_maps.append({
            "xh": xh_c, "xl": xl_c,
            "wqh": wqh, "wql": wql, "wkh": wkh, "wkl": wkl,
            "wvh": wvh, "wvl": wvl, "woh": woh, "wol": wol,
            "invu2": invu2, "ident": ident_b,
            "logb": np.ascontiguousarray(logb_c.astype(ml_dtypes.bfloat16)),
        })

    return in_maps


def kernel(x, wq, wk, wv, wo, q_norm_w, k_norm_w):
    if "nc" not in _CACHE:
        _CACHE["nc"] = _build_program()
    nc = _CACHE["nc"]
    in_maps = _make_in_maps(x, wq, wk, wv, wo, q_norm_w, k_norm_w)
    _CACHE["in_maps"] = in_maps
    import time as _time
    last_err = None
    for attempt in range(3):
        try:
            res = run_bass_kernel_spmd(nc, in_maps, core_ids=list(range(8)))
            break
        except Exception as e:  # transient NRT/device wedges recover on retry
            last_err = e
            _time.sleep(10 * (attempt + 1))
    else:
        raise last_err

    out = np.empty((B, T, DIM), dtype=np.float32)
    for c in range(8):
        b, hf = c // 2, c % 2
        out[b, hf * QTOK:(hf + 1) * QTOK, :] = res.results[c]["out"]
    return out


# revision 13
# speedup vs baseline: 1.3550x; 1.0871x over previous
"""Trainium2 Bass kernel for nn_CodecAttention (sliding-window ALiBi attention).

Reference computation (B=4, T=2048, DIM=1024, H=8, HD=128, WINDOW=16):
    xq = rms_norm(x @ wq) ; xk = rms_norm(x @ wk) ; xv = x @ wv
    scores = q k^T / sqrt(HD) + alibi_bias  (causal + 16-token sliding window)
    out = softmax(scores) @ v  -> reshape -> @ wo

Sharding: 8 cores = (batch b, sequence half). Each core processes 1024 query
tokens plus a 16-token key/value halo (zeros for the first half), fully
locally -- no collectives.

v3 design notes:
  - All four projections run as fp8e4 DoubleRow matmuls with a 3-chain hi/lo
    decomposition (x_hi*w_hi + x_lo*w_hi + x_hi*w_lo): 12 DR instructions per
    1024-deep contraction instead of 8 bf16 ones (0.75x PE cost) at
    bf16-level accuracy.  Weight hi parts are pre-scaled per tensor (SC_*);
    the scale is folded into the RMS rsqrt (q/k) or drain scale (v/wo).
  - RMS norm: drain PSUM -> bf16, squares via one DVE stt per chunk, then a
    chained ones[128,128] matmul accumulates across the 8 feature tiles AND
    broadcasts the per-token sum to all partitions; ACT Sqrt (scale folds
    SC^2, bias = eps*SC^2) + DVE reciprocal + 8 per-chunk multiplies.  The
    reduce+apply of chunk group g is emitted after group g+1's projection
    matmuls so the PE never waits on the square tiles.
  - attention per (head, 128-query tile): ALiBi+mask bias (log domain,
    masked = -30000 -> exp gives exact 0) is preloaded into PSUM via a
    144-wide ident matmul, QK accumulates on top; ACT Exp drains PSUM->SBUF
    with accum_out producing the softmax denominator for free; DVE
    reciprocal + DVE/Pool per-partition normalize; the probs transpose runs
    on the DMA xbar (dma_start_transpose of a zero-padded [128,256] tile ->
    [128,2,128]); PV in bf16; attention output drained as fp8 hi (ACT) +
    lo (DVE stt) pair.  Each query tile runs as two passes (all QK/exp
    first, then all PV/drains) so no engine queue head-of-line-blocks on
    the DMA transpose latency; wo chains and v-projection tiles are
    interleaved between passes as PE fillers.
  - output projection: 3-chain fp8 DR over (aT_hi+aT_lo)@wo_hi + aT_hi@wo_lo,
    drained with the 1/SC_O scale folded in, DMA straight to DRAM.
"""

import math
import os

import numpy as np
import ml_dtypes

os.environ.setdefault("MYCRO_LOCAL_CACHE", "1")

import concourse.mybir as mybir
import concourse.tile as tile
from concourse import bacc
from concourse.bass_utils import run_bass_kernel_spmd

F32 = mybir.dt.float32
BF16 = mybir.dt.bfloat16
F8 = mybir.dt.float8e4
AF = mybir.ActivationFunctionType
ALU = mybir.AluOpType
DR = mybir.MatmulPerfMode.DoubleRow

B, T, DIM = 4, 2048, 1024
H, HD = 8, 128
WINDOW = 16
EPS = 1e-6

HALO = 16                  # key/value halo tokens per shard
TSH = HALO + T // 2        # 1040 k/v tokens per shard
QTOK = T // 2              # 1024 query tokens per shard
ND = DIM // 128            # 8 dim tiles
NVT = 9                    # v token tiles (8*128 + 16)
NQT = QTOK // 128          # 8 query tiles
KW = 128 + HALO            # 144 keys per query tile

SC_Q = 256.0               # fp8 pre-scale for wq
SC_K = 2048.0              # fp8 pre-scale for wk*u (u ~ 1/11.3 folded in)
SC_V = 256.0               # fp8 pre-scale for wv
SC_O = 256.0               # fp8 pre-scale for wo

NEG_MASK = -30000.0        # log-domain mask; exp() underflows to exact 0

_SLOPES = [2.0 ** (-i) for i in range(H)]

_CACHE = {}


def _build_program():
    nc = bacc.Bacc("TRN2", debug=False, target_bir_lowering=False, num_devices=8)

    xh = nc.declare_dram_parameter("xh", [128, ND, TSH], F8, isOutput=False)
    xl = nc.declare_dram_parameter("xl", [128, ND, TSH], F8, isOutput=False)
    w_in = {}
    for wn in ("wq", "wk", "wv", "wo"):
        for p in ("h", "l"):
            w_in[wn + p] = nc.declare_dram_parameter(
                wn + p, [128, ND, DIM], F8, isOutput=False)
    invu2 = nc.declare_dram_parameter("invu2", [128, ND], F32, isOutput=False)
    ident = nc.declare_dram_parameter("ident", [128, 128], BF16, isOutput=False)
    logb = nc.declare_dram_parameter("logb", [128, H, 2, KW], BF16,
                                     isOutput=False)
    out = nc.declare_dram_parameter("out", [QTOK, DIM], F32, isOutput=True)

    with tile.TileContext(nc) as tc:
        _emit(tc, nc, xh, xl, w_in, invu2, ident, logb, out)
    nc.compile()
    return nc


def _emit(tc, nc, xh, xl, w_in, invu2, ident, logb, out):
    with (
        tc.tile_pool(name="big", bufs=1) as big,
        tc.tile_pool(name="sq", bufs=int(os.environ.get("KP_SQ", 18))) as sqp,
        tc.tile_pool(name="rst", bufs=4) as rstp,
        tc.tile_pool(name="ex", bufs=int(os.environ.get("KP_EX", 9))) as exp_,
        tc.tile_pool(name="exn", bufs=int(os.environ.get("KP_EXN", 9))) as exnp,
        tc.tile_pool(name="ext", bufs=int(os.environ.get("KP_EXT", 9))) as extp,
        tc.tile_pool(name="rs", bufs=10) as rsp,
        tc.tile_pool(name="at", bufs=int(os.environ.get("KP_AT", 3))) as atp,
        tc.tile_pool(name="ob", bufs=3) as obp,
        tc.tile_pool(name="pp", bufs=int(os.environ.get("KP_PP", 3)),
                     space="PSUM") as pp,
        tc.tile_pool(name="sq2", bufs=int(os.environ.get("KP_SQ2", 3)),
                     space="PSUM") as sqp2,
        tc.tile_pool(name="yt", bufs=int(os.environ.get("KP_YT", 2)),
                     space="PSUM") as ytp,
    ):
        kt_sb = big.tile([128, ND, TSH], BF16)
        qt_sb = big.tile([128, ND, QTOK], BF16)
        v_sb = big.tile([128, NVT, DIM], BF16)
        xh_sb = big.tile([128, ND, TSH], F8)
        xl_sb = big.tile([128, ND, TSH], F8)
        ws = {}
        for wn in ("wq", "wk", "wv", "wo"):
            for p in ("h", "l"):
                ws[wn + p] = big.tile([128, ND, DIM], F8, name=wn + p)
        invu2_sb = big.tile([128, ND], F32)
        ident_sb = big.tile([128, 128], BF16)
        logb_sb = big.tile([128, H, 2, KW], BF16)
        ones_sb = big.tile([128, 128], BF16)
        epsk_sb = big.tile([128, 1], F32)
        epsq_sb = big.tile([128, 1], F32)
        nc.vector.memset(ones_sb[:], 1.0)
        nc.vector.memset(epsk_sb[:], EPS * SC_K * SC_K)
        nc.vector.memset(epsq_sb[:], EPS * SC_Q * SC_Q)

        # ---- input DMAs: k weights + x first (kk-pair granularity), then
        # the rest as whole-tensor transfers.
        for j in range(ND // 2):
            nc.sync.dma_start(ws["wkh"][:, 2 * j:2 * j + 2, :],
                              w_in["wkh"][:, 2 * j:2 * j + 2, :])
            nc.sync.dma_start(xh_sb[:, 2 * j:2 * j + 2, :],
                              xh[:, 2 * j:2 * j + 2, :])
            nc.sync.dma_start(ws["wkl"][:, 2 * j:2 * j + 2, :],
                              w_in["wkl"][:, 2 * j:2 * j + 2, :])
            nc.sync.dma_start(xl_sb[:, 2 * j:2 * j + 2, :],
                              xl[:, 2 * j:2 * j + 2, :])
        nc.sync.dma_start(invu2_sb[:], invu2[:])
        nc.sync.dma_start(ident_sb[:], ident[:])
        for wn in ("wqh", "wql", "wvh", "wvl", "woh", "wol"):
            nc.sync.dma_start(ws[wn][:], w_in[wn][:])
        nc.sync.dma_start(logb_sb[:], logb[:])

        # ---------------- projection helpers ----------------
        def proj_chains(ps, wh_sb, wl_sb, m, c0, cw):
            """12 DR matmuls: xh*wh + xl*wh + xh*wl accumulated in psum."""
            for ci, (mov, sta) in enumerate(
                    [(xh_sb, wh_sb), (xl_sb, wh_sb), (xh_sb, wl_sb)]):
                for j in range(ND // 2):
                    nc.tensor.matmul(
                        ps[:, :cw],
                        sta[:, 2 * j:2 * j + 2, m * 128:(m + 1) * 128],
                        mov[:, 2 * j:2 * j + 2, c0:c0 + cw],
                        start=(ci == 0 and j == 0),
                        stop=(ci == 2 and j == ND // 2 - 1),
                        perf_mode=DR)

        def proj_group(dst, wh_sb, wl_sb, src_c0, dst_c0, cw, sq_scal, di):
            """Project one token chunk; returns square tiles for the rms."""
            sq_list = []
            for m in range(ND):
                ps = pp.tile([128, 512], F32, tag="ps", name="ps")
                proj_chains(ps, wh_sb, wl_sb, m, src_c0, cw)
                d = dst[:, m, dst_c0:dst_c0 + cw]
                if (di + m) % 2 == 0:
                    nc.scalar.copy(d, ps[:, :cw])
                else:
                    nc.vector.tensor_copy(d, ps[:, :cw])
                sq = sqp.tile([128, 512], BF16, tag="sq", name="sq")
                scal = sq_scal if isinstance(sq_scal, float) \
                    else sq_scal[:, m:m + 1]
                nc.vector.scalar_tensor_tensor(
                    out=sq[:, :cw], in0=d, scalar=scal,
                    in1=d, op0=ALU.mult, op1=ALU.mult)
                sq_list.append(sq)
            return sq_list

        def rms_reduce(dst, dst_c0, cw, sq_list, eps_sb, sc2):
            """ones-matmul partition reduce+broadcast, rsqrt, apply."""
            ssb = pp.tile([128, 512], F32, tag="ps", name="ssb")
            for m in range(ND):
                nc.tensor.matmul(ssb[:, :cw], ones_sb[:], sq_list[m][:, :cw],
                                 start=(m == 0), stop=(m == ND - 1))
            rms = rstp.tile([128, 512], F32, tag="rms", name="rms")
            nc.scalar.activation(rms[:, :cw], ssb[:, :cw], AF.Sqrt,
                                 bias=eps_sb[:], scale=sc2)
            rstd = rstp.tile([128, 512], BF16, tag="rstd", name="rstd")
            with nc.allow_low_precision(reason="bf16 rstd"):
                nc.vector.reciprocal(rstd[:, :cw], rms[:, :cw])
            for m in range(ND):
                eng = nc.gpsimd if m % 2 == 0 else nc.vector
                eng.tensor_mul(dst[:, m, dst_c0:dst_c0 + cw],
                               dst[:, m, dst_c0:dst_c0 + cw], rstd[:, :cw])

        def v_tile(tt):
            tw = 128 if tt < 8 else 16
            for nn in range(2):
                ps = pp.tile([128, 512], F32, tag="ps", name="ps")
                for ci, (sta, mov) in enumerate(
                        [(xh_sb, ws["wvh"]), (xl_sb, ws["wvh"]),
                         (xh_sb, ws["wvl"])]):
                    for j in range(ND // 2):
                        nc.tensor.matmul(
                            ps[:tw, :],
                            sta[:, 2 * j:2 * j + 2, tt * 128:tt * 128 + tw],
                            mov[:, 2 * j:2 * j + 2, nn * 512:(nn + 1) * 512],
                            start=(ci == 0 and j == 0),
                            stop=(ci == 2 and j == ND // 2 - 1),
                            perf_mode=DR)
                dstv = v_sb[:tw, tt, nn * 512:(nn + 1) * 512]
                if (tt + nn) % 2 == 0:
                    nc.scalar.activation(dstv, ps[:tw, :], AF.Copy,
                                         scale=1.0 / SC_V)
                else:
                    nc.vector.tensor_scalar_mul(dstv, ps[:tw, :], 1.0 / SC_V)

        def wo_proj(t, aTh, aTl, nn):
            ps_o = pp.tile([128, 512], F32, tag="ps", name="ps_o")
            for ci, (sta, mov) in enumerate(
                    [(aTh, ws["woh"]), (aTl, ws["woh"]), (aTh, ws["wol"])]):
                for j in range(ND // 2):
                    nc.tensor.matmul(
                        ps_o[:],
                        sta[:, 2 * j:2 * j + 2, :],
                        mov[:, 2 * j:2 * j + 2, nn * 512:(nn + 1) * 512],
                        start=(ci == 0 and j == 0),
                        stop=(ci == 2 and j == ND // 2 - 1),
                        perf_mode=DR)
            o_sb = obp.tile([128, 512], F32, tag="osb", name="o_sb")
            if nn % 2 == 0:
                nc.vector.tensor_scalar_mul(o_sb[:], ps_o[:], 1.0 / SC_O)
            else:
                nc.scalar.activation(o_sb[:], ps_o[:], AF.Copy,
                                     scale=1.0 / SC_O)
            nc.sync.dma_start(
                out[t * 128:(t + 1) * 128, nn * 512:(nn + 1) * 512],
                o_sb[:])

        # ---------------- attention tile (two passes) ----------------
        NEXN = int(os.environ.get("KP_EXN", 9))
        exn_init = [0]

        def attn_passA(t, fillers):
            """QK + exp + normalize for all 8 heads of tile t; probs of each
            head PAIR share one padded [128,512] tile so a single DMA-xbar
            transpose handles both.  `fillers` are PE-filler closures."""
            var = 0 if t == 0 else 1
            exsTs = []
            fi = 0
            for p in range(H // 2):
                exn = exnp.tile([128, 512], BF16, tag="exn", name="exn")
                if exn_init[0] < NEXN:
                    nc.vector.memset(exn[:, KW:256], 0.0)
                    nc.vector.memset(exn[:, 256 + KW:512], 0.0)
                    exn_init[0] += 1
                for i in range(2):
                    h = 2 * p + i
                    sQ_t = sqp2.tile([128, KW], F32, tag="sq", name="sQ")
                    sQ = sQ_t[:, :]
                    nc.tensor.matmul(sQ, ident_sb[:], logb_sb[:, h, var, :],
                                     start=True, stop=False)
                    nc.tensor.matmul(sQ,
                                     qt_sb[:, h, t * 128:(t + 1) * 128],
                                     kt_sb[:, h, t * 128:t * 128 + KW],
                                     start=False, stop=True)
                    ex = exp_.tile([128, KW], BF16, tag="ex", name="ex")
                    rs = rsp.tile([128, 1], F32, tag="rs", name="rs")
                    nc.scalar.activation(ex[:], sQ, AF.Exp, accum_out=rs[:])
                    rcp = rsp.tile([128, 1], F32, tag="rcp", name="rcp")
                    nc.vector.reciprocal(rcp[:], rs[:])
                    nc.gpsimd.tensor_scalar_mul(
                        exn[:, 256 * i:256 * i + KW], ex[:], rcp[:])
                exsT = extp.tile([128, 4, 128], BF16, tag="exsT", name="exsT")
                nc.sync.dma_start_transpose(exsT[:], exn[:])
                exsTs.append(exsT)
                if fi < len(fillers):
                    fillers[fi]()
                    fi += 1
            while fi < len(fillers):
                fillers[fi]()
                fi += 1
            return exsTs

        def attn_passB(t, exsTs, aTh, aTl, fillers=()):
            fi = 0
            for h in range(H):
                exsT = exsTs[h // 2]
                b0 = 2 * (h % 2)
                yT_t = ytp.tile([128, 128], F32, tag="yT", name="yT")
                yT = yT_t[:, :]
                hs = slice(h * 128, (h + 1) * 128)
                nc.tensor.matmul(yT, v_sb[:, t, hs], exsT[:, b0, :],
                                 start=True, stop=False)
                nc.tensor.matmul(yT, v_sb[0:16, t + 1, hs],
                                 exsT[0:16, b0 + 1, :], start=False, stop=True)
                with nc.allow_low_precision(reason="fp8 hi/lo attention out"):
                    if h % 2 == 0:
                        nc.scalar.copy(aTh[:, h, :], yT)
                    else:
                        nc.vector.tensor_copy(aTh[:, h, :], yT)
                    nc.vector.scalar_tensor_tensor(
                        out=aTl[:, h, :], in0=yT, scalar=1.0,
                        in1=aTh[:, h, :], op0=ALU.mult, op1=ALU.subtract)
                if h == 3 and fi < len(fillers):
                    fillers[fi]()
                    fi += 1
            while fi < len(fillers):
                fillers[fi]()
                fi += 1

        # ---------------- emission schedule ----------------
        cq = 1.0 / (SC_Q * SC_Q * DIM)
        sq_k0 = proj_group(kt_sb, ws["wkh"], ws["wkl"], 0, 0, 512, invu2_sb, 0)
        sq_k1 = proj_group(kt_sb, ws["wkh"], ws["wkl"], 512, 512, 512,
                           invu2_sb, 1)
        rms_reduce(kt_sb, 0, 512, sq_k0, epsk_sb, SC_K * SC_K)
        sq_kt = proj_group(kt_sb, ws["wkh"], ws["wkl"], 1024, 1024, 16,
                           invu2_sb, 2)
        rms_reduce(kt_sb, 512, 512, sq_k1, epsk_sb, SC_K * SC_K)
        sq_q0 = proj_group(qt_sb, ws["wqh"], ws["wql"], HALO, 0, 512, cq, 0)
        rms_reduce(kt_sb, 1024, 16, sq_kt, epsk_sb, SC_K * SC_K)
        sq_q1 = proj_group(qt_sb, ws["wqh"], ws["wql"], HALO + 512, 512, 512,
                           cq, 1)
        rms_reduce(qt_sb, 0, 512, sq_q0, epsq_sb, SC_Q * SC_Q)
        v_tile(0)
        rms_reduce(qt_sb, 512, 512, sq_q1, epsq_sb, SC_Q * SC_Q)
        v_tile(1)
        v_tile(2)
        v_tile(3)

        # software pipeline: iteration t emits passA(t), then passB(t-1);
        # wo(t-2) chains and v tiles fill PE between the QK/PV bursts.
        pend = None      # (t-1): (aT tiles, exsTs) awaiting passB
        done = None      # (t-2): aT tiles awaiting wo projection
        for t in range(NQT):
            aTh = atp.tile([128, ND, 128], F8, tag="aTh", name="aTh")
            aTl = atp.tile([128, ND, 128], F8, tag="aTl", name="aTl")
            fillA, fillB = [], []
            if done is not None:
                fillA.append(lambda t=t, p=done: wo_proj(t - 2, p[0], p[1], 0))
                fillB.append(lambda t=t, p=done: wo_proj(t - 2, p[0], p[1], 1))
            if t + 4 < NVT:
                fillA.append(lambda tt=t + 4: v_tile(tt))
            exsTs = attn_passA(t, fillA)
            if pend is not None:
                attn_passB(t - 1, pend[1], pend[0][0], pend[0][1], fillB)
            done = pend[0] if pend is not None else None
            pend = ((aTh, aTl), exsTs)
        attn_passB(NQT - 1, pend[1], pend[0][0], pend[0][1])
        wo_proj(NQT - 2, done[0], done[1], 0)
        wo_proj(NQT - 2, done[0], done[1], 1)
        wo_proj(NQT - 1, pend[0][0], pend[0][1], 0)
        wo_proj(NQT - 1, pend[0][0], pend[0][1], 1)


def _host_constants():
    # logb[i, c] = slope * (c - i - 16) inside the band (-16 <= c-i-16 <= 0),
    # else NEG_MASK.  Variant 0 additionally masks kt cols < 16 (halo before
    # sequence start).
    ii = np.arange(128)[:, None]
    cc = np.arange(KW)[None, :]
    rel = cc - ii - HALO
    band = (rel <= 0) & (rel >= -WINDOW)
    logb = np.full((128, H, 2, KW), NEG_MASK, dtype=np.float32)
    for h in range(H):
        pat = np.where(band, _SLOPES[h] * rel, NEG_MASK)
        logb[:, h, 1, :] = pat
        logb[:, h, 0, :] = np.where(cc < HALO, NEG_MASK, pat)
    ident = np.eye(128, dtype=np.float32)
    return logb, ident


def _split8(a):
    """fp8 hi/lo split of a float32 array."""
    f8 = ml_dtypes.float8_e4m3
    hi = a.astype(f8)
    lo = (a - hi.astype(np.float32)).astype(f8)
    return hi, lo


def _wlayout(w):
    """[DIM, DIM] -> [128, ND, DIM] (partition = row % 128, dim1 = row // 128)."""
    return np.ascontiguousarray(
        w.reshape(ND, 128, w.shape[1]).transpose(1, 0, 2))


def _make_in_maps(x, wq, wk, wv, wo, q_norm_w, k_norm_w):
    x = np.asarray(x, dtype=np.float32)
    wq = np.asarray(wq, dtype=np.float32)
    wk = np.asarray(wk, dtype=np.float32)
    wv = np.asarray(wv, dtype=np.float32)
    wo = np.asarray(wo, dtype=np.float32)
    q_norm_w = np.asarray(q_norm_w, dtype=np.float32)
    k_norm_w = np.asarray(k_norm_w, dtype=np.float32)

    u = (q_norm_w * k_norm_w / math.sqrt(HD)).astype(np.float32)
    wqh, wql = _split8(_wlayout(wq * SC_Q))
    wkh, wkl = _split8(_wlayout(wk * u[None, :] * SC_K))
    wvh, wvl = _split8(_wlayout(wv * SC_V))
    woh, wol = _split8(_wlayout(wo * SC_O))
    # raw sum-of-squares correction: mean_f k_raw^2 = sum_f k''^2 * invu2
    invu2 = np.ascontiguousarray(
        (1.0 / (u * u * SC_K * SC_K * DIM)).reshape(ND, 128).T
        .astype(np.float32))

    logb, ident = _host_constants()
    ident_b = ident.astype(ml_dtypes.bfloat16)

    in_maps = []
    for c in range(8):
        b, hf = c // 2, c % 2
        base = hf * (T // 2)
        xsh = np.zeros((TSH, DIM), dtype=np.float32)
        lo = base - HALO
        if lo < 0:
            xsh[HALO:] = x[b, base: base + QTOK]
        else:
            xsh[:] = x[b, lo: base + QTOK]
        xt_c = np.ascontiguousarray(
            xsh.T.reshape(ND, 128, TSH).transpose(1, 0, 2))
        xh_c, xl_c = _split8(xt_c)
        logb_c = logb.copy()
        if hf == 1:
            logb_c[:, :, 0, :] = logb_c[:, :, 1, :]
        in_maps.append({
            "xh": xh_c, "xl": xl_c,
            "wqh": wqh, "wql": wql, "wkh": wkh, "wkl": wkl,
            "wvh": wvh, "wvl": wvl, "woh": woh, "wol": wol,
            "invu2": invu2, "ident": ident_b,
            "logb": np.ascontiguousarray(logb_c.astype(ml_dtypes.bfloat16)),
        })

    return in_maps


def kernel(x, wq, wk, wv, wo, q_norm_w, k_norm_w):
    if "nc" not in _CACHE:
        _CACHE["nc"] = _build_program()
    nc = _CACHE["nc"]
    in_maps = _make_in_maps(x, wq, wk, wv, wo, q_norm_w, k_norm_w)
    _CACHE["in_maps"] = in_maps
    import time as _time
    last_err = None
    for attempt in range(3):
        try:
            res = run_bass_kernel_spmd(nc, in_maps, core_ids=list(range(8)))
            break
        except Exception as e:  # transient NRT/device wedges recover on retry
            last_err = e
            _time.sleep(10 * (attempt + 1))
    else:
        raise last_err

    out = np.empty((B, T, DIM), dtype=np.float32)
    for c in range(8):
        b, hf = c // 2, c % 2
        out[b, hf * QTOK:(hf + 1) * QTOK, :] = res.results[c]["out"]
    return out


# revision 17
# speedup vs baseline: 1.3581x; 1.0023x over previous
"""Trainium2 Bass kernel for nn_CodecAttention (sliding-window ALiBi attention).

Reference computation (B=4, T=2048, DIM=1024, H=8, HD=128, WINDOW=16):
    xq = rms_norm(x @ wq) ; xk = rms_norm(x @ wk) ; xv = x @ wv
    scores = q k^T / sqrt(HD) + alibi_bias  (causal + 16-token sliding window)
    out = softmax(scores) @ v  -> reshape -> @ wo

Sharding: 8 cores = (batch b, sequence half). Each core processes 1024 query
tokens plus a 16-token key/value halo (zeros for the first half), fully
locally -- no collectives.

v3 design notes:
  - All four projections run as fp8e4 DoubleRow matmuls with a 3-chain hi/lo
    decomposition (x_hi*w_hi + x_lo*w_hi + x_hi*w_lo): 12 DR instructions per
    1024-deep contraction instead of 8 bf16 ones (0.75x PE cost) at
    bf16-level accuracy.  Weight hi parts are pre-scaled per tensor (SC_*);
    the scale is folded into the RMS rsqrt (q/k) or drain scale (v/wo).
  - RMS norm: drain PSUM -> bf16, squares via one DVE stt per chunk, then a
    chained ones[128,128] matmul accumulates across the 8 feature tiles AND
    broadcasts the per-token sum to all partitions; ACT Sqrt (scale folds
    SC^2, bias = eps*SC^2) + DVE reciprocal + 8 per-chunk multiplies.  The
    reduce+apply of chunk group g is emitted after group g+1's projection
    matmuls so the PE never waits on the square tiles.
  - attention per (head, 128-query tile): ALiBi+mask bias (log domain,
    masked = -30000 -> exp gives exact 0) is preloaded into PSUM via a
    144-wide ident matmul, QK accumulates on top; ACT Exp drains PSUM->SBUF
    with accum_out producing the softmax denominator for free; DVE
    reciprocal + DVE/Pool per-partition normalize; the probs transpose runs
    on the DMA xbar (dma_start_transpose of a zero-padded [128,256] tile ->
    [128,2,128]); PV in bf16; attention output drained as fp8 hi (ACT) +
    lo (DVE stt) pair.  Each query tile runs as two passes (all QK/exp
    first, then all PV/drains) so no engine queue head-of-line-blocks on
    the DMA transpose latency; wo chains and v-projection tiles are
    interleaved between passes as PE fillers.
  - output projection: 3-chain fp8 DR over (aT_hi+aT_lo)@wo_hi + aT_hi@wo_lo,
    drained with the 1/SC_O scale folded in, DMA straight to DRAM.
"""

import math
import os

import numpy as np
import ml_dtypes

os.environ.setdefault("MYCRO_LOCAL_CACHE", "1")

import concourse.mybir as mybir
import concourse.tile as tile
from concourse import bacc
from concourse.bass_utils import run_bass_kernel_spmd

F32 = mybir.dt.float32
BF16 = mybir.dt.bfloat16
F8 = mybir.dt.float8e4
AF = mybir.ActivationFunctionType
ALU = mybir.AluOpType
DR = mybir.MatmulPerfMode.DoubleRow

B, T, DIM = 4, 2048, 1024
H, HD = 8, 128
WINDOW = 16
EPS = 1e-6

HALO = 16                  # key/value halo tokens per shard
TSH = HALO + T // 2        # 1040 k/v tokens per shard
QTOK = T // 2              # 1024 query tokens per shard
ND = DIM // 128            # 8 dim tiles
NVT = 9                    # v token tiles (8*128 + 16)
NQT = QTOK // 128          # 8 query tiles
KW = 128 + HALO            # 144 keys per query tile

SC_Q = 256.0               # fp8 pre-scale for wq
SC_K = 2048.0              # fp8 pre-scale for wk*u (u ~ 1/11.3 folded in)
SC_V = 256.0               # fp8 pre-scale for wv
SC_O = 256.0               # fp8 pre-scale for wo

NEG_MASK = -30000.0        # log-domain mask; exp() underflows to exact 0

_SLOPES = [2.0 ** (-i) for i in range(H)]

_CACHE = {}


def _build_program():
    nc = bacc.Bacc("TRN2", debug=False, target_bir_lowering=False, num_devices=8)

    xh = nc.declare_dram_parameter("xh", [128, ND, TSH], F8, isOutput=False)
    xl = nc.declare_dram_parameter("xl", [128, ND, TSH], F8, isOutput=False)
    w_in = {}
    for wn in ("wq", "wk", "wv", "wo"):
        for p in ("h", "l"):
            w_in[wn + p] = nc.declare_dram_parameter(
                wn + p, [128, ND, DIM], F8, isOutput=False)
    invu2 = nc.declare_dram_parameter("invu2", [128, ND], F32, isOutput=False)
    ident = nc.declare_dram_parameter("ident", [128, 128], BF16, isOutput=False)
    logb = nc.declare_dram_parameter("logb", [128, H, 2, KW], BF16,
                                     isOutput=False)
    out = nc.declare_dram_parameter("out", [QTOK, DIM], F32, isOutput=True)

    with tile.TileContext(nc) as tc:
        _emit(tc, nc, xh, xl, w_in, invu2, ident, logb, out)
    nc.compile()
    return nc


def _emit(tc, nc, xh, xl, w_in, invu2, ident, logb, out):
    with (
        tc.tile_pool(name="big", bufs=1) as big,
        tc.tile_pool(name="sq", bufs=int(os.environ.get("KP_SQ", 18))) as sqp,
        tc.tile_pool(name="rst", bufs=4) as rstp,
        tc.tile_pool(name="ex", bufs=int(os.environ.get("KP_EX", 9))) as exp_,
        tc.tile_pool(name="exn", bufs=int(os.environ.get("KP_EXN", 9))) as exnp,
        tc.tile_pool(name="ext", bufs=int(os.environ.get("KP_EXT", 9))) as extp,
        tc.tile_pool(name="rs", bufs=10) as rsp,
        tc.tile_pool(name="at", bufs=int(os.environ.get("KP_AT", 3))) as atp,
        tc.tile_pool(name="ob", bufs=3) as obp,
        tc.tile_pool(name="pp", bufs=int(os.environ.get("KP_PP", 3)),
                     space="PSUM") as pp,
        tc.tile_pool(name="sq2", bufs=int(os.environ.get("KP_SQ2", 3)),
                     space="PSUM") as sqp2,
        tc.tile_pool(name="yt", bufs=int(os.environ.get("KP_YT", 2)),
                     space="PSUM") as ytp,
    ):
        kt_sb = big.tile([128, ND, TSH], BF16)
        qt_sb = big.tile([128, ND, QTOK], BF16)
        v_sb = big.tile([128, NVT, DIM], BF16)
        xh_sb = big.tile([128, ND, TSH], F8)
        xl_sb = big.tile([128, ND, TSH], F8)
        ws = {}
        for wn in ("wq", "wk", "wv", "wo"):
            for p in ("h", "l"):
                ws[wn + p] = big.tile([128, ND, DIM], F8, name=wn + p)
        invu2_sb = big.tile([128, ND], F32)
        ident_sb = big.tile([128, 128], BF16)
        logb_sb = big.tile([128, H, 2, KW], BF16)
        ones_sb = big.tile([128, 128], BF16)
        epsk_sb = big.tile([128, 1], F32)
        epsq_sb = big.tile([128, 1], F32)
        nc.vector.memset(ones_sb[:], 1.0)
        nc.vector.memset(epsk_sb[:], EPS * SC_K * SC_K)
        nc.vector.memset(epsq_sb[:], EPS * SC_Q * SC_Q)

        # ---- input DMAs: k weights + x first (kk-pair granularity), then
        # the rest as whole-tensor transfers.
        # order matches chain consumption: xh*wkh first, then xl, then wkl
        for j in range(ND // 2):
            nc.sync.dma_start(ws["wkh"][:, 2 * j:2 * j + 2, :],
                              w_in["wkh"][:, 2 * j:2 * j + 2, :])
            nc.sync.dma_start(xh_sb[:, 2 * j:2 * j + 2, :],
                              xh[:, 2 * j:2 * j + 2, :])
        for j in range(ND // 2):
            nc.sync.dma_start(xl_sb[:, 2 * j:2 * j + 2, :],
                              xl[:, 2 * j:2 * j + 2, :])
        for j in range(ND // 2):
            nc.sync.dma_start(ws["wkl"][:, 2 * j:2 * j + 2, :],
                              w_in["wkl"][:, 2 * j:2 * j + 2, :])
        nc.sync.dma_start(invu2_sb[:], invu2[:])
        nc.sync.dma_start(ident_sb[:], ident[:])
        for wn in ("wqh", "wql", "wvh", "wvl", "woh", "wol"):
            nc.sync.dma_start(ws[wn][:], w_in[wn][:])
        nc.sync.dma_start(logb_sb[:], logb[:])

        # ---------------- projection helpers ----------------
        def proj_chains(ps, wh_sb, wl_sb, m, c0, cw):
            """12 DR matmuls: xh*wh + xl*wh + xh*wl accumulated in psum."""
            for ci, (mov, sta) in enumerate(
                    [(xh_sb, wh_sb), (xl_sb, wh_sb), (xh_sb, wl_sb)]):
                for j in range(ND // 2):
                    nc.tensor.matmul(
                        ps[:, :cw],
                        sta[:, 2 * j:2 * j + 2, m * 128:(m + 1) * 128],
                        mov[:, 2 * j:2 * j + 2, c0:c0 + cw],
                        start=(ci == 0 and j == 0),
                        stop=(ci == 2 and j == ND // 2 - 1),
                        perf_mode=DR)

        def proj_group(dst, wh_sb, wl_sb, src_c0, dst_c0, cw, sq_scal, di):
            """Project one token chunk; returns square tiles for the rms."""
            sq_list = []
            for m in range(ND):
                ps = pp.tile([128, 512], F32, tag="ps", name="ps")
                proj_chains(ps, wh_sb, wl_sb, m, src_c0, cw)
                d = dst[:, m, dst_c0:dst_c0 + cw]
                if (di + m) % 2 == 0:
                    nc.scalar.copy(d, ps[:, :cw])
                else:
                    nc.vector.tensor_copy(d, ps[:, :cw])
                sq = sqp.tile([128, 512], BF16, tag="sq", name="sq")
                scal = sq_scal if isinstance(sq_scal, float) \
                    else sq_scal[:, m:m + 1]
                nc.vector.scalar_tensor_tensor(
                    out=sq[:, :cw], in0=d, scalar=scal,
                    in1=d, op0=ALU.mult, op1=ALU.mult)
                sq_list.append(sq)
            return sq_list

        def rms_reduce(dst, dst_c0, cw, sq_list, eps_sb, sc2):
            """ones-matmul partition reduce+broadcast, rsqrt, apply."""
            ssb = pp.tile([128, 512], F32, tag="ps", name="ssb")
            for m in range(ND):
                nc.tensor.matmul(ssb[:, :cw], ones_sb[:], sq_list[m][:, :cw],
                                 start=(m == 0), stop=(m == ND - 1))
            rms = rstp.tile([128, 512], F32, tag="rms", name="rms")
            nc.scalar.activation(rms[:, :cw], ssb[:, :cw], AF.Sqrt,
                                 bias=eps_sb[:], scale=sc2)
            rstd = rstp.tile([128, 512], BF16, tag="rstd", name="rstd")
            with nc.allow_low_precision(reason="bf16 rstd"):
                nc.vector.reciprocal(rstd[:, :cw], rms[:, :cw])
            for m in range(ND):
                eng = nc.gpsimd if m % 2 == 0 else nc.vector
                eng.tensor_mul(dst[:, m, dst_c0:dst_c0 + cw],
                               dst[:, m, dst_c0:dst_c0 + cw], rstd[:, :cw])

        def v_tile(tt, nns=(0, 1)):
            tw = 128 if tt < 8 else 16
            for nn in nns:
                ps = pp.tile([128, 512], F32, tag="ps", name="ps")
                for ci, (sta, mov) in enumerate(
                        [(xh_sb, ws["wvh"]), (xl_sb, ws["wvh"]),
                         (xh_sb, ws["wvl"])]):
                    for j in range(ND // 2):
                        nc.tensor.matmul(
                            ps[:tw, :],
                            sta[:, 2 * j:2 * j + 2, tt * 128:tt * 128 + tw],
                            mov[:, 2 * j:2 * j + 2, nn * 512:(nn + 1) * 512],
                            start=(ci == 0 and j == 0),
                            stop=(ci == 2 and j == ND // 2 - 1),
                            perf_mode=DR)
                dstv = v_sb[:tw, tt, nn * 512:(nn + 1) * 512]
                if (tt + nn) % 2 == 0:
                    nc.scalar.activation(dstv, ps[:tw, :], AF.Copy,
                                         scale=1.0 / SC_V)
                else:
                    nc.vector.tensor_scalar_mul(dstv, ps[:tw, :], 1.0 / SC_V)

        def wo_proj(t, aTh, aTl, nn):
            ps_o = pp.tile([128, 512], F32, tag="ps", name="ps_o")
            for ci, (sta, mov) in enumerate(
                    [(aTh, ws["woh"]), (aTl, ws["woh"]), (aTh, ws["wol"])]):
                for j in range(ND // 2):
                    nc.tensor.matmul(
                        ps_o[:],
                        sta[:, 2 * j:2 * j + 2, :],
                        mov[:, 2 * j:2 * j + 2, nn * 512:(nn + 1) * 512],
                        start=(ci == 0 and j == 0),
                        stop=(ci == 2 and j == ND // 2 - 1),
                        perf_mode=DR)
            o_sb = obp.tile([128, 512], F32, tag="osb", name="o_sb")
            if nn % 2 == 0:
                nc.vector.tensor_scalar_mul(o_sb[:], ps_o[:], 1.0 / SC_O)
            else:
                nc.scalar.activation(o_sb[:], ps_o[:], AF.Copy,
                                     scale=1.0 / SC_O)
            nc.sync.dma_start(
                out[t * 128:(t + 1) * 128, nn * 512:(nn + 1) * 512],
                o_sb[:])

        # ---------------- attention tile (two passes) ----------------
        NEXN = int(os.environ.get("KP_EXN", 9))
        exn_init = [0]

        def attn_passA(t, fillers):
            """QK + exp + normalize for all 8 heads of tile t; probs of each
            head PAIR share one padded [128,512] tile so a single DMA-xbar
            transpose handles both.  `fillers` are PE-filler closures."""
            var = 0 if t == 0 else 1
            exsTs = []
            fi = 0
            for p in range(H // 2):
                exn = exnp.tile([128, 512], BF16, tag="exn", name="exn")
                if exn_init[0] < NEXN:
                    nc.vector.memset(exn[:, KW:256], 0.0)
                    nc.vector.memset(exn[:, 256 + KW:512], 0.0)
                    exn_init[0] += 1
                for i in range(2):
                    h = 2 * p + i
                    sQ_t = sqp2.tile([128, KW], F32, tag="sq", name="sQ")
                    sQ = sQ_t[:, :]
                    nc.tensor.matmul(sQ, ident_sb[:], logb_sb[:, h, var, :],
                                     start=True, stop=False)
                    nc.tensor.matmul(sQ,
                                     qt_sb[:, h, t * 128:(t + 1) * 128],
                                     kt_sb[:, h, t * 128:t * 128 + KW],
                                     start=False, stop=True)
                    ex = exp_.tile([128, KW], BF16, tag="ex", name="ex")
                    rs = rsp.tile([128, 1], F32, tag="rs", name="rs")
                    nc.scalar.activation(ex[:], sQ, AF.Exp, accum_out=rs[:])
                    rcp = rsp.tile([128, 1], F32, tag="rcp", name="rcp")
                    nc.vector.reciprocal(rcp[:], rs[:])
                    nc.gpsimd.tensor_scalar_mul(
                        exn[:, 256 * i:256 * i + KW], ex[:], rcp[:])
                exsT = extp.tile([128, 4, 128], BF16, tag="exsT", name="exsT")
                nc.sync.dma_start_transpose(exsT[:], exn[:])
                exsTs.append(exsT)
                if fi < len(fillers):
                    fillers[fi]()
                    fi += 1
            while fi < len(fillers):
                fillers[fi]()
                fi += 1
            return exsTs

        def attn_passB(t, exsTs, aTh, aTl, fillers=()):
            fi = 0
            for h in range(H):
                exsT = exsTs[h // 2]
                b0 = 2 * (h % 2)
                yT_t = ytp.tile([128, 128], F32, tag="yT", name="yT")
                yT = yT_t[:, :]
                hs = slice(h * 128, (h + 1) * 128)
                nc.tensor.matmul(yT, v_sb[:, t, hs], exsT[:, b0, :],
                                 start=True, stop=False)
                nc.tensor.matmul(yT, v_sb[0:16, t + 1, hs],
                                 exsT[0:16, b0 + 1, :], start=False, stop=True)
                with nc.allow_low_precision(reason="fp8 hi/lo attention out"):
                    if h % 2 == 0:
                        nc.scalar.copy(aTh[:, h, :], yT)
                    else:
                        nc.vector.tensor_copy(aTh[:, h, :], yT)
                    nc.vector.scalar_tensor_tensor(
                        out=aTl[:, h, :], in0=yT, scalar=1.0,
                        in1=aTh[:, h, :], op0=ALU.mult, op1=ALU.subtract)
                if h == 3 and fi < len(fillers):
                    fillers[fi]()
                    fi += 1
            while fi < len(fillers):
                fillers[fi]()
                fi += 1

        # ---------------- emission schedule ----------------
        cq = 1.0 / (SC_Q * SC_Q * DIM)
        sq_k0 = proj_group(kt_sb, ws["wkh"], ws["wkl"], 0, 0, 512, invu2_sb, 0)
        sq_k1 = proj_group(kt_sb, ws["wkh"], ws["wkl"], 512, 512, 512,
                           invu2_sb, 1)
        rms_reduce(kt_sb, 0, 512, sq_k0, epsk_sb, SC_K * SC_K)
        sq_kt = proj_group(kt_sb, ws["wkh"], ws["wkl"], 1024, 1024, 16,
                           invu2_sb, 2)
        rms_reduce(kt_sb, 512, 512, sq_k1, epsk_sb, SC_K * SC_K)
        sq_q0 = proj_group(qt_sb, ws["wqh"], ws["wql"], HALO, 0, 512, cq, 0)
        rms_reduce(kt_sb, 1024, 16, sq_kt, epsk_sb, SC_K * SC_K)
        sq_q1 = proj_group(qt_sb, ws["wqh"], ws["wql"], HALO + 512, 512, 512,
                           cq, 1)
        rms_reduce(qt_sb, 0, 512, sq_q0, epsq_sb, SC_Q * SC_Q)
        v_tile(0)
        rms_reduce(qt_sb, 512, 512, sq_q1, epsq_sb, SC_Q * SC_Q)
        v_tile(1)
        v_tile(2)
        v_tile(3)

        # software pipeline: iteration t emits passA(t), then passB(t-1);
        # wo(t-2) chains and v-tile halves fill PE between the QK/PV bursts.
        # v tile j must be fully emitted by iteration j (passB(j-1) reads it).
        V_HALVES = {0: [(4, 0)], 1: [(4, 1)], 2: [(5, 0)],
                    3: [(5, 1), (6, 0)], 4: [(6, 1)], 5: [(7, 0)],
                    6: [(7, 1), (8, 0)], 7: [(8, 1)]}
        pend = None      # (t-1): (aT tiles, exsTs) awaiting passB
        done = None      # (t-2): aT tiles awaiting wo projection
        for t in range(NQT):
            aTh = atp.tile([128, ND, 128], F8, tag="aTh", name="aTh")
            aTl = atp.tile([128, ND, 128], F8, tag="aTl", name="aTl")
            fillA, fillB = [], []
            if done is not None:
                fillA.append(lambda t=t, p=done: wo_proj(t - 2, p[0], p[1], 0))
                fillB.append(lambda t=t, p=done: wo_proj(t - 2, p[0], p[1], 1))
            for tt, nn in V_HALVES.get(t, ()):
                fillA.append(lambda tt=tt, nn=nn: v_tile(tt, (nn,)))
            exsTs = attn_passA(t, fillA)
            if pend is not None:
                attn_passB(t - 1, pend[1], pend[0][0], pend[0][1], fillB)
            done = pend[0] if pend is not None else None
            pend = ((aTh, aTl), exsTs)
        attn_passB(NQT - 1, pend[1], pend[0][0], pend[0][1], [
            lambda: wo_proj(NQT - 2, done[0], done[1], 0),
            lambda: wo_proj(NQT - 2, done[0], done[1], 1),
        ])
        wo_proj(NQT - 1, pend[0][0], pend[0][1], 0)
        wo_proj(NQT - 1, pend[0][0], pend[0][1], 1)


def _host_constants():
    # logb[i, c] = slope * (c - i - 16) inside the band (-16 <= c-i-16 <= 0),
    # else NEG_MASK.  Variant 0 additionally masks kt cols < 16 (halo before
    # sequence start).
    ii = np.arange(128)[:, None]
    cc = np.arange(KW)[None, :]
    rel = cc - ii - HALO
    band = (rel <= 0) & (rel >= -WINDOW)
    logb = np.full((128, H, 2, KW), NEG_MASK, dtype=np.float32)
    for h in range(H):
        pat = np.where(band, _SLOPES[h] * rel, NEG_MASK)
        logb[:, h, 1, :] = pat
        logb[:, h, 0, :] = np.where(cc < HALO, NEG_MASK, pat)
    ident = np.eye(128, dtype=np.float32)
    return logb, ident


def _split8(a):
    """fp8 hi/lo split of a float32 array."""
    f8 = ml_dtypes.float8_e4m3
    hi = a.astype(f8)
    lo = (a - hi.astype(np.float32)).astype(f8)
    return hi, lo


def _wlayout(w):
    """[DIM, DIM] -> [128, ND, DIM] (partition = row % 128, dim1 = row // 128)."""
    return np.ascontiguousarray(
        w.reshape(ND, 128, w.shape[1]).transpose(1, 0, 2))


def _make_in_maps(x, wq, wk, wv, wo, q_norm_w, k_norm_w):
    x = np.asarray(x, dtype=np.float32)
    wq = np.asarray(wq, dtype=np.float32)
    wk = np.asarray(wk, dtype=np.float32)
    wv = np.asarray(wv, dtype=np.float32)
    wo = np.asarray(wo, dtype=np.float32)
    q_norm_w = np.asarray(q_norm_w, dtype=np.float32)
    k_norm_w = np.asarray(k_norm_w, dtype=np.float32)

    u = (q_norm_w * k_norm_w / math.sqrt(HD)).astype(np.float32)
    wqh, wql = _split8(_wlayout(wq * SC_Q))
    wkh, wkl = _split8(_wlayout(wk * u[None, :] * SC_K))
    wvh, wvl = _split8(_wlayout(wv * SC_V))
    woh, wol = _split8(_wlayout(wo * SC_O))
    # raw sum-of-squares correction: mean_f k_raw^2 = sum_f k''^2 * invu2
    invu2 = np.ascontiguousarray(
        (1.0 / (u * u * SC_K * SC_K * DIM)).reshape(ND, 128).T
        .astype(np.float32))

    logb, ident = _host_constants()
    ident_b = ident.astype(ml_dtypes.bfloat16)

    in_maps = []
    for c in range(8):
        b, hf = c // 2, c % 2
        base = hf * (T // 2)
        xsh = np.zeros((TSH, DIM), dtype=np.float32)
        lo = base - HALO
        if lo < 0:
            xsh[HALO:] = x[b, base: base + QTOK]
        else:
            xsh[:] = x[b, lo: base + QTOK]
        xt_c = np.ascontiguousarray(
            xsh.T.reshape(ND, 128, TSH).transpose(1, 0, 2))
        xh_c, xl_c = _split8(xt_c)
        logb_c = logb.copy()
        if hf == 1:
            logb_c[:, :, 0, :] = logb_c[:, :, 1, :]
        in_maps.append({
            "xh": xh_c, "xl": xl_c,
            "wqh": wqh, "wql": wql, "wkh": wkh, "wkl": wkl,
            "wvh": wvh, "wvl": wvl, "woh": woh, "wol": wol,
            "invu2": invu2, "ident": ident_b,
            "logb": np.ascontiguousarray(logb_c.astype(ml_dtypes.bfloat16)),
        })

    return in_maps


def kernel(x, wq, wk, wv, wo, q_norm_w, k_norm_w):
    if "nc" not in _CACHE:
        _CACHE["nc"] = _build_program()
    nc = _CACHE["nc"]
    in_maps = _make_in_maps(x, wq, wk, wv, wo, q_norm_w, k_norm_w)
    _CACHE["in_maps"] = in_maps
    import time as _time
    last_err = None
    for attempt in range(3):
        try:
            res = run_bass_kernel_spmd(nc, in_maps, core_ids=list(range(8)))
            break
        except Exception as e:  # transient NRT/device wedges recover on retry
            last_err = e
            _time.sleep(10 * (attempt + 1))
    else:
        raise last_err

    out = np.empty((B, T, DIM), dtype=np.float32)
    for c in range(8):
        b, hf = c // 2, c % 2
        out[b, hf * QTOK:(hf + 1) * QTOK, :] = res.results[c]["out"]
    return out


# revision 21
# speedup vs baseline: 1.5427x; 1.1360x over previous
"""Trainium2 Bass kernel for nn_CodecAttention (sliding-window ALiBi attention).

Reference computation (B=4, T=2048, DIM=1024, H=8, HD=128, WINDOW=16):
    xq = rms_norm(x @ wq) ; xk = rms_norm(x @ wk) ; xv = x @ wv
    scores = q k^T / sqrt(HD) + alibi_bias  (causal + 16-token sliding window)
    out = softmax(scores) @ v  -> reshape -> @ wo

Sharding: 8 cores = (batch b, sequence half). Each core processes 1024 query
tokens plus a 16-token key/value halo (zeros for the first half), fully
locally -- no collectives.

v3 design notes:
  - All four projections run as fp8e4 DoubleRow matmuls with a 3-chain hi/lo
    decomposition (x_hi*w_hi + x_lo*w_hi + x_hi*w_lo): 12 DR instructions per
    1024-deep contraction instead of 8 bf16 ones (0.75x PE cost) at
    bf16-level accuracy.  Weight hi parts are pre-scaled per tensor (SC_*);
    the scale is folded into the RMS rsqrt (q/k) or drain scale (v/wo).
  - RMS norm: drain PSUM -> bf16, squares via one DVE stt per chunk, then a
    chained ones[128,128] matmul accumulates across the 8 feature tiles AND
    broadcasts the per-token sum to all partitions; ACT Sqrt (scale folds
    SC^2, bias = eps*SC^2) + DVE reciprocal + 8 per-chunk multiplies.  The
    reduce+apply of chunk group g is emitted after group g+1's projection
    matmuls so the PE never waits on the square tiles.
  - attention per (head, 128-query tile): ALiBi+mask bias (log domain,
    masked = -30000 -> exp gives exact 0) is preloaded into PSUM via a
    144-wide ident matmul, QK accumulates on top; ACT Exp drains PSUM->SBUF
    with accum_out producing the softmax denominator for free; DVE
    reciprocal + DVE/Pool per-partition normalize; the probs transpose runs
    on the DMA xbar (dma_start_transpose of a zero-padded [128,256] tile ->
    [128,2,128]); PV in bf16; attention output drained as fp8 hi (ACT) +
    lo (DVE stt) pair.  Each query tile runs as two passes (all QK/exp
    first, then all PV/drains) so no engine queue head-of-line-blocks on
    the DMA transpose latency; wo chains and v-projection tiles are
    interleaved between passes as PE fillers.
  - output projection: 3-chain fp8 DR over (aT_hi+aT_lo)@wo_hi + aT_hi@wo_lo,
    drained with the 1/SC_O scale folded in, DMA straight to DRAM.
"""

import math
import os

import numpy as np
import ml_dtypes

os.environ.setdefault("MYCRO_LOCAL_CACHE", "1")

import concourse.mybir as mybir
import concourse.tile as tile
from concourse import bacc
from concourse.bass_utils import run_bass_kernel_spmd

F32 = mybir.dt.float32
BF16 = mybir.dt.bfloat16
F8 = mybir.dt.float8e4
AF = mybir.ActivationFunctionType
ALU = mybir.AluOpType
DR = mybir.MatmulPerfMode.DoubleRow

B, T, DIM = 4, 2048, 1024
H, HD = 8, 128
WINDOW = 16
EPS = 1e-6

HALO = 16                  # key/value halo tokens per shard
TSH = HALO + T // 2        # 1040 k/v tokens per shard
QTOK = T // 2              # 1024 query tokens per shard
ND = DIM // 128            # 8 dim tiles
NVT = 9                    # v token tiles (8*128 + 16)
NQT = QTOK // 128          # 8 query tiles
KW = 128 + HALO            # 144 keys per query tile

SC_Q = 256.0               # fp8 pre-scale for wq
SC_K = 2048.0              # fp8 pre-scale for wk*u (u ~ 1/11.3 folded in)
SC_V = 256.0               # fp8 pre-scale for wv
SC_O = 256.0               # fp8 pre-scale for wo

NEG_MASK = -30000.0        # log-domain mask; exp() underflows to exact 0

_SLOPES = [2.0 ** (-i) for i in range(H)]

_CACHE = {}


def _build_program():
    nc = bacc.Bacc("TRN2", debug=False, target_bir_lowering=False, num_devices=8)

    xh = nc.declare_dram_parameter("xh", [128, ND, TSH], F8, isOutput=False)
    xl = nc.declare_dram_parameter("xl", [128, ND, TSH], F8, isOutput=False)
    w_in = {}
    for wn in ("wq", "wk", "wv", "wo"):
        for p in ("h", "l"):
            w_in[wn + p] = nc.declare_dram_parameter(
                wn + p, [128, ND, DIM], F8, isOutput=False)
    invu2 = nc.declare_dram_parameter("invu2", [128, ND], F32, isOutput=False)
    ident = nc.declare_dram_parameter("ident", [128, 128], BF16, isOutput=False)
    logb = nc.declare_dram_parameter("logb", [128, H, 2, KW], BF16,
                                     isOutput=False)
    out = nc.declare_dram_parameter("out", [QTOK, DIM], F32, isOutput=True)

    with tile.TileContext(nc) as tc:
        _emit(tc, nc, xh, xl, w_in, invu2, ident, logb, out)
    nc.compile()
    return nc


def _emit(tc, nc, xh, xl, w_in, invu2, ident, logb, out):
    with (
        tc.tile_pool(name="big", bufs=1) as big,
        tc.tile_pool(name="sq", bufs=int(os.environ.get("KP_SQ", 18))) as sqp,
        tc.tile_pool(name="rst", bufs=4) as rstp,
        tc.tile_pool(name="ex", bufs=int(os.environ.get("KP_EX", 9))) as exp_,
        tc.tile_pool(name="exn", bufs=int(os.environ.get("KP_EXN", 9))) as exnp,
        tc.tile_pool(name="ext", bufs=int(os.environ.get("KP_EXT", 9))) as extp,
        tc.tile_pool(name="rs", bufs=10) as rsp,
        tc.tile_pool(name="at", bufs=int(os.environ.get("KP_AT", 3))) as atp,
        tc.tile_pool(name="ob", bufs=4) as obp,
        tc.tile_pool(name="pp", bufs=int(os.environ.get("KP_PP", 3)),
                     space="PSUM") as pp,
        tc.tile_pool(name="sq2", bufs=int(os.environ.get("KP_SQ2", 3)),
                     space="PSUM") as sqp2,
        tc.tile_pool(name="yt", bufs=int(os.environ.get("KP_YT", 2)),
                     space="PSUM") as ytp,
    ):
        kt_sb = big.tile([128, ND, TSH], BF16)
        qt_sb = big.tile([128, ND, QTOK], BF16)
        v_sb = big.tile([128, NVT, DIM], BF16)
        xh_sb = big.tile([128, ND, TSH], F8)
        xl_sb = big.tile([128, ND, TSH], F8)
        ws = {}
        for wn in ("wq", "wk", "wv", "wo"):
            for p in ("h", "l"):
                ws[wn + p] = big.tile([128, ND, DIM], F8, name=wn + p)
        invu2_sb = big.tile([128, ND], F32)
        ident_sb = big.tile([128, 128], BF16)
        logb_sb = big.tile([128, H, 2, KW], BF16)
        ones_sb = big.tile([128, 128], BF16)
        epsk_sb = big.tile([128, 1], F32)
        epsq_sb = big.tile([128, 1], F32)
        nc.vector.memset(ones_sb[:], 1.0)
        nc.vector.memset(epsk_sb[:], EPS * SC_K * SC_K)
        nc.vector.memset(epsq_sb[:], EPS * SC_Q * SC_Q)

        # ---- input DMAs: k weights + x first (kk-pair granularity), then
        # the rest as whole-tensor transfers.
        # order matches chain consumption: xh*wkh first, then xl, then wkl
        for j in range(ND // 2):
            nc.sync.dma_start(ws["wkh"][:, 2 * j:2 * j + 2, :],
                              w_in["wkh"][:, 2 * j:2 * j + 2, :])
            nc.sync.dma_start(xh_sb[:, 2 * j:2 * j + 2, :],
                              xh[:, 2 * j:2 * j + 2, :])
        for j in range(ND // 2):
            nc.sync.dma_start(xl_sb[:, 2 * j:2 * j + 2, :],
                              xl[:, 2 * j:2 * j + 2, :])
        for j in range(ND // 2):
            nc.sync.dma_start(ws["wkl"][:, 2 * j:2 * j + 2, :],
                              w_in["wkl"][:, 2 * j:2 * j + 2, :])
        nc.sync.dma_start(invu2_sb[:], invu2[:])
        nc.sync.dma_start(ident_sb[:], ident[:])
        for wn in ("wqh", "wql", "wvh", "wvl", "woh", "wol"):
            nc.sync.dma_start(ws[wn][:], w_in[wn][:])
        nc.sync.dma_start(logb_sb[:], logb[:])

        # ---------------- projection helpers ----------------
        def proj_chains(ps, wh_sb, wl_sb, m, c0, cw):
            """12 DR matmuls: xh*wh + xl*wh + xh*wl accumulated in psum."""
            for ci, (mov, sta) in enumerate(
                    [(xh_sb, wh_sb), (xl_sb, wh_sb), (xh_sb, wl_sb)]):
                for j in range(ND // 2):
                    nc.tensor.matmul(
                        ps[:, :cw],
                        sta[:, 2 * j:2 * j + 2, m * 128:(m + 1) * 128],
                        mov[:, 2 * j:2 * j + 2, c0:c0 + cw],
                        start=(ci == 0 and j == 0),
                        stop=(ci == 2 and j == ND // 2 - 1),
                        perf_mode=DR)

        def proj_group(dst, wh_sb, wl_sb, src_c0, dst_c0, cw, sq_scal, di):
            """Project one token chunk; returns square tiles for the rms."""
            sq_list = []
            for m in range(ND):
                ps = pp.tile([128, 512], F32, tag="ps", name="ps")
                proj_chains(ps, wh_sb, wl_sb, m, src_c0, cw)
                d = dst[:, m, dst_c0:dst_c0 + cw]
                if (di + m) % 2 == 0:
                    nc.scalar.copy(d, ps[:, :cw])
                else:
                    nc.vector.tensor_copy(d, ps[:, :cw])
                sq = sqp.tile([128, 512], BF16, tag="sq", name="sq")
                scal = sq_scal if isinstance(sq_scal, float) \
                    else sq_scal[:, m:m + 1]
                nc.vector.scalar_tensor_tensor(
                    out=sq[:, :cw], in0=d, scalar=scal,
                    in1=d, op0=ALU.mult, op1=ALU.mult)
                sq_list.append(sq)
            return sq_list

        def rms_reduce(dst, dst_c0, cw, sq_list, eps_sb, sc2):
            """ones-matmul partition reduce+broadcast, rsqrt, apply."""
            ssb = pp.tile([128, 512], F32, tag="ps", name="ssb")
            for m in range(ND):
                nc.tensor.matmul(ssb[:, :cw], ones_sb[:], sq_list[m][:, :cw],
                                 start=(m == 0), stop=(m == ND - 1))
            rms = rstp.tile([128, 512], F32, tag="rms", name="rms")
            nc.scalar.activation(rms[:, :cw], ssb[:, :cw], AF.Sqrt,
                                 bias=eps_sb[:], scale=sc2)
            rstd = rstp.tile([128, 512], BF16, tag="rstd", name="rstd")
            with nc.allow_low_precision(reason="bf16 rstd"):
                nc.vector.reciprocal(rstd[:, :cw], rms[:, :cw])
            for m in range(ND):
                eng = nc.gpsimd if m % 2 == 0 else nc.vector
                eng.tensor_mul(dst[:, m, dst_c0:dst_c0 + cw],
                               dst[:, m, dst_c0:dst_c0 + cw], rstd[:, :cw])

        def v_tile(tt, nns=(0, 1)):
            tw = 128 if tt < 8 else 16
            for nn in nns:
                ps = pp.tile([128, 512], F32, tag="ps", name="ps")
                for ci, (sta, mov) in enumerate(
                        [(xh_sb, ws["wvh"]), (xl_sb, ws["wvh"]),
                         (xh_sb, ws["wvl"])]):
                    for j in range(ND // 2):
                        nc.tensor.matmul(
                            ps[:tw, :],
                            sta[:, 2 * j:2 * j + 2, tt * 128:tt * 128 + tw],
                            mov[:, 2 * j:2 * j + 2, nn * 512:(nn + 1) * 512],
                            start=(ci == 0 and j == 0),
                            stop=(ci == 2 and j == ND // 2 - 1),
                            perf_mode=DR)
                dstv = v_sb[:tw, tt, nn * 512:(nn + 1) * 512]
                if tt < 4 and (tt + nn) % 2 == 0:
                    nc.scalar.activation(dstv, ps[:tw, :], AF.Copy,
                                         scale=1.0 / SC_V)
                else:
                    # keep the ACT queue clear for exps once attention starts
                    nc.vector.tensor_scalar_mul(dstv, ps[:tw, :], 1.0 / SC_V)

        out_pend = []    # deferred (t, nn, o_sb) output DMAs

        def wo_proj(t, aTh, aTl, nn):
            ps_o = pp.tile([128, 512], F32, tag="ps", name="ps_o")
            for ci, (sta, mov) in enumerate(
                    [(aTh, ws["woh"]), (aTl, ws["woh"]), (aTh, ws["wol"])]):
                for j in range(ND // 2):
                    nc.tensor.matmul(
                        ps_o[:],
                        sta[:, 2 * j:2 * j + 2, :],
                        mov[:, 2 * j:2 * j + 2, nn * 512:(nn + 1) * 512],
                        start=(ci == 0 and j == 0),
                        stop=(ci == 2 and j == ND // 2 - 1),
                        perf_mode=DR)
            o_sb = obp.tile([128, 512], F32, tag="osb", name="o_sb")
            nc.vector.tensor_scalar_mul(o_sb[:], ps_o[:], 1.0 / SC_O)
            out_pend.append((t, nn, o_sb))

        def flush_out():
            # emit output DMAs in a batch so they never sit between the
            # latency-critical probs-transpose issues on the SP queue
            while out_pend:
                t, nn, o_sb = out_pend.pop(0)
                nc.sync.dma_start(
                    out[t * 128:(t + 1) * 128, nn * 512:(nn + 1) * 512],
                    o_sb[:])

        # ---------------- attention tile (two passes) ----------------
        NEXN = int(os.environ.get("KP_EXN", 9))
        exn_init = [0]

        def attn_passA(t, fillers):
            """QK + exp + normalize for all 8 heads of tile t; probs of each
            head PAIR share one padded [128,512] tile so a single DMA-xbar
            transpose handles both.  `fillers` are PE-filler closures."""
            var = 0 if t == 0 else 1
            exsTs = []
            fi = 0
            for p in range(H // 2):
                exn = exnp.tile([128, 512], BF16, tag="exn", name="exn")
                if exn_init[0] < NEXN:
                    nc.vector.memset(exn[:, KW:256], 0.0)
                    nc.vector.memset(exn[:, 256 + KW:512], 0.0)
                    exn_init[0] += 1
                for i in range(2):
                    h = 2 * p + i
                    sQ_t = sqp2.tile([128, KW], F32, tag="sq", name="sQ")
                    sQ = sQ_t[:, :]
                    nc.tensor.matmul(sQ, ident_sb[:], logb_sb[:, h, var, :],
                                     start=True, stop=False)
                    nc.tensor.matmul(sQ,
                                     qt_sb[:, h, t * 128:(t + 1) * 128],
                                     kt_sb[:, h, t * 128:t * 128 + KW],
                                     start=False, stop=True)
                    ex = exp_.tile([128, KW], BF16, tag="ex", name="ex")
                    rs = rsp.tile([128, 1], F32, tag="rs", name="rs")
                    nc.scalar.activation(ex[:], sQ, AF.Exp, accum_out=rs[:])
                    rcp = rsp.tile([128, 1], F32, tag="rcp", name="rcp")
                    nc.vector.reciprocal(rcp[:], rs[:])
                    nc.gpsimd.tensor_scalar_mul(
                        exn[:, 256 * i:256 * i + KW], ex[:], rcp[:])
                exsT = extp.tile([128, 4, 128], BF16, tag="exsT", name="exsT")
                nc.sync.dma_start_transpose(exsT[:], exn[:])
                exsTs.append(exsT)
                if fi < len(fillers):
                    fillers[fi]()
                    fi += 1
            while fi < len(fillers):
                fillers[fi]()
                fi += 1
            return exsTs

        def attn_passB(t, exsTs, aTh, aTl, fillers=()):
            fi = 0
            for h in range(H):
                exsT = exsTs[h // 2]
                b0 = 2 * (h % 2)
                yT_t = ytp.tile([128, 128], F32, tag="yT", name="yT")
                yT = yT_t[:, :]
                hs = slice(h * 128, (h + 1) * 128)
                nc.tensor.matmul(yT, v_sb[:, t, hs], exsT[:, b0, :],
                                 start=True, stop=False)
                nc.tensor.matmul(yT, v_sb[0:16, t + 1, hs],
                                 exsT[0:16, b0 + 1, :], start=False, stop=True)
                with nc.allow_low_precision(reason="fp8 hi/lo attention out"):
                    if h % 2 == 0:
                        nc.scalar.copy(aTh[:, h, :], yT)
                    else:
                        nc.vector.tensor_copy(aTh[:, h, :], yT)
                    nc.vector.scalar_tensor_tensor(
                        out=aTl[:, h, :], in0=yT, scalar=1.0,
                        in1=aTh[:, h, :], op0=ALU.mult, op1=ALU.subtract)
                if h == 3 and fi < len(fillers):
                    fillers[fi]()
                    fi += 1
            while fi < len(fillers):
                fillers[fi]()
                fi += 1

        # ---------------- emission schedule ----------------
        cq = 1.0 / (SC_Q * SC_Q * DIM)
        sq_k0 = proj_group(kt_sb, ws["wkh"], ws["wkl"], 0, 0, 512, invu2_sb, 0)
        sq_k1 = proj_group(kt_sb, ws["wkh"], ws["wkl"], 512, 512, 512,
                           invu2_sb, 1)
        rms_reduce(kt_sb, 0, 512, sq_k0, epsk_sb, SC_K * SC_K)
        sq_kt = proj_group(kt_sb, ws["wkh"], ws["wkl"], 1024, 1024, 16,
                           invu2_sb, 2)
        rms_reduce(kt_sb, 512, 512, sq_k1, epsk_sb, SC_K * SC_K)
        sq_q0 = proj_group(qt_sb, ws["wqh"], ws["wql"], HALO, 0, 512, cq, 0)
        rms_reduce(kt_sb, 1024, 16, sq_kt, epsk_sb, SC_K * SC_K)
        sq_q1 = proj_group(qt_sb, ws["wqh"], ws["wql"], HALO + 512, 512, 512,
                           cq, 1)
        rms_reduce(qt_sb, 0, 512, sq_q0, epsq_sb, SC_Q * SC_Q)
        v_tile(0)
        rms_reduce(qt_sb, 512, 512, sq_q1, epsq_sb, SC_Q * SC_Q)
        v_tile(1)
        v_tile(2)
        v_tile(3)

        # software pipeline: iteration t emits passA(t), then passB(t-1);
        # wo(t-2) chains and v-tile halves fill PE between the QK/PV bursts.
        # v tile j must be fully emitted by iteration j (passB(j-1) reads it).
        V_HALVES = {0: [(4, 0)], 1: [(4, 1)], 2: [(5, 0)],
                    3: [(5, 1), (6, 0)], 4: [(6, 1)], 5: [(7, 0)],
                    6: [(7, 1), (8, 0)], 7: [(8, 1)]}
        pend = None      # (t-1): (aT tiles, exsTs) awaiting passB
        done = None      # (t-2): aT tiles awaiting wo projection
        for t in range(NQT):
            aTh = atp.tile([128, ND, 128], F8, tag="aTh", name="aTh")
            aTl = atp.tile([128, ND, 128], F8, tag="aTl", name="aTl")
            fillA, fillB = [], []
            if done is not None:
                fillA.append(lambda t=t, p=done: wo_proj(t - 2, p[0], p[1], 0))
                fillB.append(lambda t=t, p=done: wo_proj(t - 2, p[0], p[1], 1))
            for tt, nn in V_HALVES.get(t, ()):
                fillA.append(lambda tt=tt, nn=nn: v_tile(tt, (nn,)))
            exsTs = attn_passA(t, fillA)
            if pend is not None:
                attn_passB(t - 1, pend[1], pend[0][0], pend[0][1], fillB)
            flush_out()
            done = pend[0] if pend is not None else None
            pend = ((aTh, aTl), exsTs)
        attn_passB(NQT - 1, pend[1], pend[0][0], pend[0][1], [
            lambda: wo_proj(NQT - 2, done[0], done[1], 0),
            lambda: wo_proj(NQT - 2, done[0], done[1], 1),
        ])
        wo_proj(NQT - 1, pend[0][0], pend[0][1], 0)
        wo_proj(NQT - 1, pend[0][0], pend[0][1], 1)
        flush_out()


def _host_constants():
    # logb[i, c] = slope * (c - i - 16) inside the band (-16 <= c-i-16 <= 0),
    # else NEG_MASK.  Variant 0 additionally masks kt cols < 16 (halo before
    # sequence start).
    ii = np.arange(128)[:, None]
    cc = np.arange(KW)[None, :]
    rel = cc - ii - HALO
    band = (rel <= 0) & (rel >= -WINDOW)
    logb = np.full((128, H, 2, KW), NEG_MASK, dtype=np.float32)
    for h in range(H):
        pat = np.where(band, _SLOPES[h] * rel, NEG_MASK)
        logb[:, h, 1, :] = pat
        logb[:, h, 0, :] = np.where(cc < HALO, NEG_MASK, pat)
    ident = np.eye(128, dtype=np.float32)
    return logb, ident


def _split8(a):
    """fp8 hi/lo split of a float32 array."""
    f8 = ml_dtypes.float8_e4m3
    hi = a.astype(f8)
    lo = (a - hi.astype(np.float32)).astype(f8)
    return hi, lo


def _wlayout(w):
    """[DIM, DIM] -> [128, ND, DIM] (partition = row % 128, dim1 = row // 128)."""
    return np.ascontiguousarray(
        w.reshape(ND, 128, w.shape[1]).transpose(1, 0, 2))


def _make_in_maps(x, wq, wk, wv, wo, q_norm_w, k_norm_w):
    x = np.asarray(x, dtype=np.float32)
    wq = np.asarray(wq, dtype=np.float32)
    wk = np.asarray(wk, dtype=np.float32)
    wv = np.asarray(wv, dtype=np.float32)
    wo = np.asarray(wo, dtype=np.float32)
    q_norm_w = np.asarray(q_norm_w, dtype=np.float32)
    k_norm_w = np.asarray(k_norm_w, dtype=np.float32)

    u = (q_norm_w * k_norm_w / math.sqrt(HD)).astype(np.float32)
    wqh, wql = _split8(_wlayout(wq * SC_Q))
    wkh, wkl = _split8(_wlayout(wk * u[None, :] * SC_K))
    wvh, wvl = _split8(_wlayout(wv * SC_V))
    woh, wol = _split8(_wlayout(wo * SC_O))
    # raw sum-of-squares correction: mean_f k_raw^2 = sum_f k''^2 * invu2
    invu2 = np.ascontiguousarray(
        (1.0 / (u * u * SC_K * SC_K * DIM)).reshape(ND, 128).T
        .astype(np.float32))

    logb, ident = _host_constants()
    ident_b = ident.astype(ml_dtypes.bfloat16)

    in_maps = []
    for c in range(8):
        b, hf = c // 2, c % 2
        base = hf * (T // 2)
        xsh = np.zeros((TSH, DIM), dtype=np.float32)
        lo = base - HALO
        if lo < 0:
            xsh[HALO:] = x[b, base: base + QTOK]
        else:
            xsh[:] = x[b, lo: base + QTOK]
        xt_c = np.ascontiguousarray(
            xsh.T.reshape(ND, 128, TSH).transpose(1, 0, 2))
        xh_c, xl_c = _split8(xt_c)
        logb_c = logb.copy()
        if hf == 1:
            logb_c[:, :, 0, :] = logb_c[:, :, 1, :]
        in_maps.append({
            "xh": xh_c, "xl": xl_c,
            "wqh": wqh, "wql": wql, "wkh": wkh, "wkl": wkl,
            "wvh": wvh, "wvl": wvl, "woh": woh, "wol": wol,
            "invu2": invu2, "ident": ident_b,
            "logb": np.ascontiguousarray(logb_c.astype(ml_dtypes.bfloat16)),
        })

    return in_maps


def kernel(x, wq, wk, wv, wo, q_norm_w, k_norm_w):
    if "nc" not in _CACHE:
        _CACHE["nc"] = _build_program()
    nc = _CACHE["nc"]
    in_maps = _make_in_maps(x, wq, wk, wv, wo, q_norm_w, k_norm_w)
    _CACHE["in_maps"] = in_maps
    import time as _time
    last_err = None
    for attempt in range(3):
        try:
            res = run_bass_kernel_spmd(nc, in_maps, core_ids=list(range(8)))
            break
        except Exception as e:  # transient NRT/device wedges recover on retry
            last_err = e
            _time.sleep(10 * (attempt + 1))
    else:
        raise last_err

    out = np.empty((B, T, DIM), dtype=np.float32)
    for c in range(8):
        b, hf = c // 2, c % 2
        out[b, hf * QTOK:(hf + 1) * QTOK, :] = res.results[c]["out"]
    return out
